# revision 88
# baseline (speedup 1.0000x reference)
"""Trainium2 Bass kernel for nn_DependencyNeuralModel (dependency parser scorer).

v2 design (8 NeuronCores, SPMD):
  Encoder: 2-layer BiLSTM over S=512, replicated on every core, chunk-parallel
    (64 chunks x 2 dirs advance lock-step as 128 rows through the PE).
    K_WARM=16 warmup steps; gate order repacked to [i,f,o,g] so the i/f
    half of the recurrent GEMM can overlap the o/g half's activations.
  Arc scores: score(h,m) depends only on the (h,m) pair (dist is a function
    of m-h), so each core builds the 64-row slice of the full SxS score
    table it owns (h sharded), in a transposed layout where the dist term
    is a contiguous slice of a host-built [H, 1023] offset table and the
    head term is a per-partition activation-fused bias.  The per-arc gather
    is then a single GPSIMD ap_gather of scalar (pair) entries from the
    partition-replicated table; host picks the parity lane and unsorts.
  Sib scores: part-sharded; host sorts each core's 16384 parts by the
    (head,mod,sib) 128-chunk combo (64 combos x 3 static tiles), so each
    128-part tile needs only 3 one-hot gather matmuls instead of 12.
Host does only index/layout preparation and final unshard.
"""
import sys
import types

import numpy as np

sys.path.insert(0, "/opt/trn_rl_repo")

import concourse.bass as bass
import concourse.mybir as mybir
from concourse.tile import TileContext
from concourse.masks import make_identity

S = 512
H = 512
A = 262144
ASIB = 131072
NB = 17
L = 8
K_WARM = 14
NSTEP = K_WARM + L  # 24
NC = 8
F32 = mybir.dt.float32
BF16 = mybir.dt.bfloat16
U16 = mybir.dt.uint16
BINS = np.array(list(range(10)) + list(range(10, 40, 5)) + [40], dtype=np.int64)

GPERM = np.r_[0:1024, 1536:2048, 1024:1536]  # gate reorder i,f,g,o -> i,f,o,g


def _install_ntff_hook():
    if "antenv.axon_hooks" in sys.modules:
        return
    mod = types.ModuleType("antenv.axon_hooks")
    state = {"hook": None, "tried": False}

    def set_axon_ntff_profile_hook(hook):
        state["hook"] = hook

    def get_axon_ntff_profile_hook():
        if state["hook"] is None and not state["tried"]:
            state["tried"] = True
            try:
                from trn_agent_boot.trn_boot import _ntff_profile_via_ctypes

                state["hook"] = _ntff_profile_via_ctypes("/opt/axon/libaxon_pjrt.so")
            except Exception:
                state["hook"] = None
        return state["hook"]

    mod.set_axon_ntff_profile_hook = set_axon_ntff_profile_hook
    mod.get_axon_ntff_profile_hook = get_axon_ntff_profile_hook
    import antenv

    antenv.axon_hooks = mod
    sys.modules["antenv.axon_hooks"] = mod


def _legalize_waits(nc):
    """This walrus accepts at most one semaphore wait per instruction;
    split extra waits onto same-engine NOPs placed just before."""
    ctr = [0]
    for f in nc.m.functions:
        for blk in f.blocks:
            out = []
            dirty = False
            for ins in blk.instructions:
                si = ins.sync_info
                if si is not None and si.on_wait and len(si.on_wait) > 1:
                    waits = list(si.on_wait)
                    for w in waits[:-1]:
                        ctr[0] += 1
                        nop = mybir.InstNoOp(name=f"waitfix-{ctr[0]}")
                        nop.engine = ins.engine
                        nop.sync_info = mybir.SyncInfo(on_wait=[w], on_update=[])
                        out.append(nop)
                    ins.sync_info = mybir.SyncInfo(
                        on_wait=[waits[-1]],
                        on_update=list(si.on_update) if si.on_update else [],
                    )
                    dirty = True
                out.append(ins)
            if dirty:
                blk.instructions = out
    return nc


def _lstm_layer(nc, tc, ident, mask_sb, whhT_dram, wx_dram, dstT, dstTrev,
                whh_pre=None):
    """One BiLSTM layer, chunk-parallel.  B=128 rows: partitions 0:64 are
    dir0 chunks, 64:128 dir1 chunks.  Gate columns are [i,f,o,g]; the
    i/f half of the recurrent GEMM is emitted first so its activations
    overlap the o/g half's matmuls.  Output states are written directly
    into the feature-major [128, 9, 512] tiles dstT (and optionally the
    position-reversed dstTrev) from the per-step transposed state."""
    import contextlib

    with contextlib.ExitStack() as ctx:
        sb = ctx.enter_context(tc.tile_pool(name="lstm_sb", bufs=4))
        cold = ctx.enter_context(tc.tile_pool(name="lstm_cold", bufs=1))
        st = ctx.enter_context(tc.tile_pool(name="lstm_state", bufs=1))
        ps1 = ctx.enter_context(tc.tile_pool(name="lstm_ps1", bufs=1, space="PSUM"))
        ps2 = ctx.enter_context(tc.tile_pool(name="lstm_ps2", bufs=1, space="PSUM"))
        pst = ctx.enter_context(tc.tile_pool(name="lstm_pst", bufs=2, space="PSUM"))

        if whh_pre is not None:
            whh_sb = whh_pre
        else:
            whh_sb = st.tile([128, 4, 2, 2048], BF16)
            nc.sync.dma_start(
                whh_sb.rearrange("p a b c -> p (a b c)"),
                whhT_dram.rearrange("p a b c -> p (a b c)"),
            )
        h_t = st.tile([128, 4, 128], BF16)  # h transposed: [k-part, kc, b]
        c_st = st.tile([128, 512], BF16)    # [b, k]
        nc.vector.memset(h_t.rearrange("p a b -> p (a b)"), 0.0)
        nc.vector.memset(c_st[:], 0.0)

        def fetch_wx(s):
            wx = sb.tile([128, 2048], BF16, tag="wx")
            for d in range(2):
                nc.sync.dma_start(
                    wx[d * 64:(d + 1) * 64, :], wx_dram[d, s:s + 505:8, :]
                )
            return wx

        def emit_ident(wx):
            """identity-injection matmuls; emitted during the previous
            step's tail so the PE fills otherwise-idle cycles."""
            g01 = ps1.tile([128, 1024], F32, tag="g01")
            g23 = ps2.tile([128, 1024], F32, tag="g23")
            for half, gps in ((0, g01), (1, g23)):
                for ng in range(2):
                    col = (half * 2 + ng) * 512
                    for d in range(2):
                        bs = slice(d * 64, (d + 1) * 64)
                        nc.tensor.matmul(
                            gps[bs, ng * 512:(ng + 1) * 512],
                            lhsT=ident[:, bs],
                            rhs=wx[:, col:col + 512],
                            start=True, stop=False,
                        )
            return g01, g23

        wxq = {0: fetch_wx(0), 1: fetch_wx(1), 2: fetch_wx(2)}
        nxt = emit_ident(wxq.pop(0))
        for s in range(NSTEP):
            # recurrent half of the gates GEMM.  d0 writes psum rows 0:64
            # (array cols 0-63), d1 rows 64:128 (cols 64-127); adjacent
            # d0/d1 matmuls run concurrently as (128,64) column tiles.
            g01, g23 = nxt
            for half, gps in ((0, g01), (1, g23)):
                for ng in range(2):
                    col = (half * 2 + ng) * 512
                    for kc in range(4):
                        for d in range(2):
                            bs = slice(d * 64, (d + 1) * 64)
                            nc.tensor.matmul(
                                gps[bs, ng * 512:(ng + 1) * 512],
                                lhsT=h_t[:, kc, bs],
                                rhs=whh_sb[:, kc, d, col:col + 512],
                                start=False,
                                stop=(kc == 3),
                            )
            sig_if = cold.tile([128, 1024], BF16, tag="sif")
            nc.scalar.activation(sig_if[:], g01[:],
                                 mybir.ActivationFunctionType.Sigmoid)
            tanh_g = cold.tile([128, 512], BF16, tag="tg")
            nc.scalar.activation(tanh_g[:], g23[:, 512:1024],
                                 mybir.ActivationFunctionType.Tanh)
            sig_o = cold.tile([128, 512], BF16, tag="so")
            nc.scalar.activation(sig_o[:], g23[:, 0:512],
                                 mybir.ActivationFunctionType.Sigmoid)
            t1 = cold.tile([128, 512], BF16, tag="t1")
            nc.vector.tensor_mul(t1[:], sig_if[:, 512:1024], c_st[:])
            t2 = cold.tile([128, 512], BF16, tag="t2")
            nc.vector.tensor_mul(t2[:], sig_if[:, 0:512], tanh_g[:])
            nc.vector.tensor_add(c_st[:], t1[:], t2[:])
            tch = cold.tile([128, 512], BF16, tag="tch")
            nc.scalar.activation(tch[:], c_st[:], mybir.ActivationFunctionType.Tanh)
            h_new = cold.tile([128, 512], BF16, tag="h")
            nc.vector.tensor_mul(h_new[:], sig_o[:], tch[:])
            if s in (K_WARM - 9, K_WARM - 1):
                mi = {K_WARM - 9: 0, K_WARM - 1: 1}[s]
                nc.vector.tensor_scalar_mul(h_new[:], h_new[:], mask_sb[:, mi:mi + 1])
                nc.vector.tensor_scalar_mul(c_st[:], c_st[:], mask_sb[:, mi:mi + 1])
            if s + 3 < NSTEP:
                wxq[s + 3] = fetch_wx(s + 3)
            if s + 1 < NSTEP:
                # next step's identity matmuls go into the PE queue BEFORE
                # this step's transposes: they have no h dependency and run
                # while the tail above executes.
                nxt = emit_ident(wxq.pop(s + 1))
            tp = pst.tile([128, 4, 128], BF16, tag="tr_ps")
            for kc in range(4):
                nc.tensor.transpose(tp[:, kc, :],
                                    h_new[:, kc * 128:(kc + 1) * 128], ident[:])
            nc.vector.tensor_copy(h_t.rearrange("p a b -> p (a b)"),
                                  tp.rearrange("p a b -> p (a b)"))
            if s >= K_WARM:
                o = s - K_WARM
                # scatter this step's transposed states into the
                # feature-major output tiles, which use the PERMUTED
                # position order p' = (pos%8)*64 + pos//8 so every write
                # is a contiguous 64-column block.  dir0 chunk c is
                # position 8c+o -> block o; dir1 (backward) chunk c is
                # position 511-(8c+o) -> block 7-o, chunk axis reversed.
                nc.vector.tensor_copy(dstT[:, 0:4, o * 64:(o + 1) * 64],
                                      tp[:, :, 0:64])
                nc.vector.tensor_copy(
                    dstT[:, 4:8, (7 - o) * 64:(8 - o) * 64],
                    tp[:, :, 127:63:-1])
                if dstTrev is not None:
                    nc.vector.tensor_copy(
                        dstTrev[:, 0:4, (7 - o) * 64:(8 - o) * 64],
                        tp[:, :, 63::-1])
                    nc.vector.tensor_copy(
                        dstTrev[:, 4:8, o * 64:(o + 1) * 64],
                        tp[:, :, 64:128])


def _input_gemm(nc, tc, lhsT_tiles, wihT_dram, wx_dram, nk, klast, pre=None):
    """WX[d] = lhsT_d.T @ wihT[d] -> wx_dram[d, K_WARM:K_WARM+512, :].
    lhsT_tiles: per-dir tile [128, nk, 512] in SBUF ([feat-part, chunk, pos]).
    nk chunks; last chunk has klast valid rows.  If pre is given it is an
    SBUF-resident [128, nk, 2, 2048] copy of the weights (prefetched long
    before, so this GEMM issues no DMA reads at all)."""
    import contextlib

    with contextlib.ExitStack() as ctx:
        sb = ctx.enter_context(tc.tile_pool(name="ig_sb", bufs=6))
        ps = ctx.enter_context(tc.tile_pool(name="ig_ps", bufs=2, space="PSUM"))
        for d in range(2):
            lhsT = lhsT_tiles[d]
            for ngc in range(4):
                acc4 = ps.tile([128, 4, 512], F32, tag="acc4")
                for kc in range(nk):
                    kk = 128 if kc < nk - 1 else klast
                    if pre is not None:
                        rhs_ap = pre[:kk, kc, d, ngc * 512:(ngc + 1) * 512]
                    else:
                        rhs = sb.tile([128, 512], wihT_dram.dtype, tag="rhs")
                        nc.sync.dma_start(
                            rhs[:kk, :],
                            wihT_dram[kc * 128:kc * 128 + kk, d,
                                      ngc * 512:(ngc + 1) * 512],
                        )
                        rhs_ap = rhs[:kk, :]
                    for mc in range(4):
                        nc.tensor.matmul(
                            acc4[:, mc, :],
                            lhsT=lhsT[:kk, kc, mc * 128:(mc + 1) * 128],
                            rhs=rhs_ap,
                            start=(kc == 0),
                            stop=(kc == nk - 1),
                        )
                osb = sb.tile([128, 4, 512], BF16, tag="osb")
                nc.scalar.activation(
                    osb.rearrange("p a b -> p (a b)"),
                    acc4.rearrange("p a b -> p (a b)"),
                    mybir.ActivationFunctionType.Copy)
                # lhsT columns are in permuted position order
                # p' = o*64 + c (o = 2*mc + a); scatter rows back to the
                # natural sliding-window rows 8c + o of wx_dram.
                for mc in range(4):
                    for a in range(2):
                        r0 = K_WARM + 2 * mc + a
                        nc.sync.dma_start(
                            wx_dram[d, r0:r0 + 505:8,
                                    ngc * 512:(ngc + 1) * 512],
                            osb[a * 64:(a + 1) * 64, mc, :],
                        )


def _build(nc, sib_combos, arc_buckets):
    dt = F32
    n_sib_tile = len(sib_combos)       # even
    n_arc_tile = len(arc_buckets)      # even
    n_tile = n_sib_tile + n_arc_tile
    embT_f = nc.dram_tensor("embT_f", [128, 3, 512], BF16, kind="ExternalInput")
    embT_b = nc.dram_tensor("embT_b", [128, 3, 512], BF16, kind="ExternalInput")
    wih0T = nc.dram_tensor("wih0T", [384, 2, 2048], BF16, kind="ExternalInput")
    whh0T = nc.dram_tensor("whh0T", [128, 4, 2, 2048], BF16, kind="ExternalInput")
    wih1T = nc.dram_tensor("wih1T", [1152, 2, 2048], BF16, kind="ExternalInput")
    whh1T = nc.dram_tensor("whh1T", [128, 4, 2, 2048], BF16, kind="ExternalInput")
    projT = nc.dram_tensor("projT", [1152, 2560], BF16, kind="ExternalInput")
    dwin_in = nc.dram_tensor("dwin_in", [128, 4, 576], BF16, kind="ExternalInput")
    hsel_in = nc.dram_tensor("hsel_in", [128, 4, 64], BF16, kind="ExternalInput")
    wrep_in = nc.dram_tensor("wrep_in", [128, 512], BF16, kind="ExternalInput")
    wrepT_in = nc.dram_tensor("wrepT_in", [128, 4, 128], BF16, kind="ExternalInput")
    sib_oh_in = nc.dram_tensor("sib_oh_in", [n_sib_tile // 2, 128, 768], BF16,
                               kind="ExternalInput")
    arc_oh_in = nc.dram_tensor("arc_oh_in", [n_arc_tile // 2, 64, 256], dt,
                               kind="ExternalInput")
    arcm_in = nc.dram_tensor("arcm_in", [128, n_arc_tile], dt,
                             kind="ExternalInput")
    iotar_in = nc.dram_tensor("iotar_in", [128, 128], dt, kind="ExternalInput")
    mask_in = nc.dram_tensor("mask_in", [128, 2], dt, kind="ExternalInput")
    scores_out = nc.dram_tensor("scores_out", [128, n_tile], dt,
                                kind="ExternalOutput")

    wx0 = nc.dram_tensor("wx0", [2, 544, 2048], BF16)
    tdram = nc.dram_tensor("tdram", [64, 512], F32)
    wx1 = nc.dram_tensor("wx1", [2, 544, 2048], BF16)
    tdram = nc.dram_tensor("tdram", [64, 512], F32)

    import contextlib

    with TileContext(nc) as tc:
        with contextlib.ExitStack() as ctx:
            const = ctx.enter_context(tc.tile_pool(name="const", bufs=1))
            big = ctx.enter_context(tc.tile_pool(name="big", bufs=1))

            ident = const.tile([128, 128], BF16)
            make_identity(nc, ident[:])
            mask_sb = const.tile([128, 2], dt)
            nc.sync.dma_start(mask_sb[:], mask_in[:])
            one_row = const.tile([1, 512], BF16)
            nc.vector.memset(one_row[:], 1.0)
            wrep_sb = const.tile([128, 512], BF16)
            wrepT_sb = const.tile([128, 4, 128], BF16)
            dwin_sb = const.tile([128, 4, 576], BF16)
            hsel_sb = const.tile([128, 4, 64], BF16)
            iota_row = const.tile([128, 128], dt)
            arcm_sb = const.tile([128, n_arc_tile], dt)

            # zero-pad warmup rows of WX buffers
            with tc.tile_pool(name="zp", bufs=1) as zp:
                zrow = zp.tile([64, 2048], BF16)
                nc.vector.memset(zrow[:], 0.0)
                for wxd in (wx0, wx1):
                    for d in range(2):
                        nc.sync.dma_start(wxd[d, 0:K_WARM, :], zrow[0:K_WARM, :])
                        nc.sync.dma_start(wxd[d, K_WARM + 512:544, :],
                                          zrow[0:32 - K_WARM, :])

            # ---- layer 0 (streams states into x1T / x1Trev) ----
            x1T = big.tile([128, 9, 512], BF16, tag="x1T")
            x1Trev = big.tile([128, 9, 512], BF16, tag="x1Trev")
            for dst in (x1T, x1Trev):
                nc.vector.memset(dst[:, 8, :], 0.0)
                nc.vector.tensor_copy(dst[0:1, 8, :], one_row[:])

            with tc.tile_pool(name="w1pre", bufs=1) as w1p:
                # ---- WX0 + weight prefetch ----
                # the sync queue stalls at WX0's first sem-gated output
                # write, so any DMA issued after WX0 starts ~30us late.
                # Issue embeddings first, then layer-0 recurrent weights,
                # then the wih1T prefetch -- all BEFORE the WX0 GEMM body.
                with tc.tile_pool(name="emb_sb", bufs=1) as emb_pool:
                    ef = emb_pool.tile([128, 3, 512], BF16)
                    nc.sync.dma_start(ef.rearrange("p a b -> p (a b)"),
                                      embT_f.rearrange("p a b -> p (a b)"))
                    eb = emb_pool.tile([128, 3, 512], BF16)
                    nc.sync.dma_start(eb.rearrange("p a b -> p (a b)"),
                                      embT_b.rearrange("p a b -> p (a b)"))
                    whh_sb = big.tile([128, 4, 2, 2048], BF16, tag="whh")
                    nc.sync.dma_start(
                        whh_sb.rearrange("p a b c -> p (a b c)"),
                        whh0T.rearrange("p a b c -> p (a b c)"))
                    _input_gemm(nc, tc, [ef, eb], wih0T, wx0, 3, 128)
                # wih1T prefetch: queued behind WX0's sync-stalls is fine --
                # it only has to complete before WX1, ~280us later.
                w1sb = w1p.tile([128, 9, 2, 2048], BF16, tag="w1")
                for kc in range(9):
                    for d in range(2):
                        nc.sync.dma_start(
                            w1sb[:, kc, d, :],
                            wih1T[kc * 128:(kc + 1) * 128, d, :])

                _lstm_layer(nc, tc, ident, mask_sb, whh0T, wx0, x1T, x1Trev,
                            whh_pre=whh_sb)

                # reload the shared recurrent-weight tile with layer 1's
                # weights; transfers during WX1 so layer 1 starts instantly
                nc.sync.dma_start(
                    whh_sb.rearrange("p a b c -> p (a b c)"),
                    whh1T.rearrange("p a b c -> p (a b c)"))

                # scoring-only constants load behind the startup-critical DMAs
                nc.sync.dma_start(wrep_sb[:], wrep_in[:])
                nc.sync.dma_start(wrepT_sb.rearrange("p a b -> p (a b)"),
                                  wrepT_in.rearrange("p a b -> p (a b)"))
                nc.sync.dma_start(dwin_sb.rearrange("p a b -> p (a b)"),
                                  dwin_in.rearrange("p a b -> p (a b)"))
                nc.sync.dma_start(hsel_sb.rearrange("p a b -> p (a b)"),
                                  hsel_in.rearrange("p a b -> p (a b)"))
                nc.sync.dma_start(iota_row[:], iotar_in[:])
                nc.sync.dma_start(arcm_sb[:], arcm_in[:])

                # ---- WX1 ----
                _input_gemm(nc, tc, [x1T, x1Trev], wih1T, wx1, 9, 1, pre=w1sb)

            # ---- layer 1 (streams states into stT) ----
            stT = big.tile([128, 9, 512], BF16, tag="x1Trev")  # reuse slot
            nc.vector.memset(stT[:, 8, :], 0.0)
            nc.vector.tensor_copy(stT[0:1, 8, :], one_row[:])
            ppre_pool = ctx.enter_context(tc.tile_pool(name="ppre", bufs=1))
            ppre = ppre_pool.tile([128, 9, 2560], BF16)
            for kc in range(9):
                nc.sync.dma_start(ppre[:, kc, :],
                                  projT[kc * 128:(kc + 1) * 128, :])
            _lstm_layer(nc, tc, ident, mask_sb, whh1T, wx1, stT, None,
                        whh_pre=whh_sb)

            # ---- pos-major projection tables (head + 3 sib; skip mod) ----
            tables_sb = big.tile([128, 4, 2560], BF16, tag="tables")
            with contextlib.ExitStack() as c2:
                sb2 = c2.enter_context(tc.tile_pool(name="tb_sb", bufs=6))
                with tc.tile_pool(name="tb_ps4", bufs=2, space="PSUM") as ps4:
                    for ngc in (0,):
                        acc4 = ps4.tile([128, 4, 512], dt, tag="acc4")
                        for kc in range(9):
                            kk = 128 if kc < 8 else 1
                            for mc in range(4):
                                nc.tensor.matmul(
                                    acc4[:, mc, :],
                                    lhsT=stT[:kk, kc, mc * 128:(mc + 1) * 128],
                                    rhs=ppre[:kk, kc,
                                             ngc * 512:(ngc + 1) * 512],
                                    start=(kc == 0),
                                    stop=(kc == 8),
                                )
                        for mc in range(4):
                            nc.scalar.activation(
                                tables_sb[:, mc, ngc * 512:(ngc + 1) * 512],
                                acc4[:, mc, :],
                                mybir.ActivationFunctionType.Copy)
                ps2 = c2.enter_context(tc.tile_pool(name="tb_ps", bufs=2,
                                                    space="PSUM"))

                # ---- transposed mod table M_T[j, m] ----
                mTp = big.tile([128, 4, 512], BF16, tag="mTp")
                for jc in range(4):
                    acc = ps2.tile([128, 512], dt, tag="acc")
                    for kc in range(8):
                        nc.tensor.matmul(
                            acc[:],
                            lhsT=ppre[:, kc,
                                      512 + jc * 128:512 + (jc + 1) * 128],
                            rhs=stT[:, kc, :],
                            start=(kc == 0), stop=(kc == 7),
                        )
                    nc.scalar.activation(mTp[:, jc, :], acc[:],
                                         mybir.ActivationFunctionType.Copy)
                # un-permute m columns to natural position order (one-time)
                mT = big.tile([128, 4, 512], BF16, tag="mT")
                for o in range(8):
                    nc.vector.tensor_copy(mT[:, :, o::8],
                                          mTp[:, :, o * 64:(o + 1) * 64])

                # ---- H window: hwin[j, hl] = heads[64c+hl, j] ----
                hwin = big.tile([128, 4, 64], dt, tag="hwin")
                for jc in range(4):
                    acc = ps2.tile([128, 64], dt, tag="acch")
                    for kc in range(4):
                        nc.tensor.matmul(
                            acc[:],
                            lhsT=tables_sb[:, kc, jc * 128:(jc + 1) * 128],
                            rhs=hsel_sb[:, kc, :],
                            start=(kc == 0), stop=(kc == 3),
                        )
                    nc.vector.tensor_copy(hwin[:, jc, :], acc[:])

                # sib projection tables AFTER mT/hwin: the scoring rows only
                # need mT+hwin, so their ACT/DVE pipeline starts while these
                # PE-only GEMMs still stream.
                with tc.tile_pool(name="tb_ps5", bufs=1, space="PSUM") as ps5:
                    for ngc in (2, 3, 4):
                        acc4 = ps5.tile([128, 4, 512], dt, tag="acc4b")
                        for kc in range(9):
                            kk = 128 if kc < 8 else 1
                            for mc in range(4):
                                nc.tensor.matmul(
                                    acc4[:, mc, :],
                                    lhsT=stT[:kk, kc, mc * 128:(mc + 1) * 128],
                                    rhs=ppre[:kk, kc,
                                             ngc * 512:(ngc + 1) * 512],
                                    start=(kc == 0),
                                    stop=(kc == 8),
                                )
                        for mc in range(4):
                            nc.scalar.activation(
                                tables_sb[:, mc, ngc * 512:(ngc + 1) * 512],
                                acc4[:, mc, :],
                                mybir.ActivationFunctionType.Copy)

            # ---- scoring ----
            scores_sb = big.tile([128, n_tile], dt, tag="scores")
            with contextlib.ExitStack() as c3:
                sb3 = c3.enter_context(tc.tile_pool(name="sc_sb", bufs=4))
                ps_sacc = c3.enter_context(tc.tile_pool(name="ps_sacc", bufs=2,
                                                        space="PSUM"))

                def sib_pair(p):
                    # sib tiles 2p, 2p+1 with host-uploaded one-hots
                    oh = sb3.tile([128, 2, 3, 128], BF16, tag="oh")
                    nc.sync.dma_start(
                        oh.rearrange("p a b c -> p (a b c)"),
                        sib_oh_in[p, :, :],
                    )
                    sacc2 = ps_sacc.tile([128, 2, 512], dt, tag="sacc2")
                    for h2 in range(2):
                        combo = sib_combos[2 * p + h2]
                        chunks = (combo // 16, (combo // 4) % 4, combo % 4)
                        for g in range(3):
                            nc.tensor.matmul(
                                sacc2[:, h2, :], lhsT=oh[:, h2, g, :],
                                rhs=tables_sb[:, chunks[g],
                                              1024 + g * 512:
                                              1024 + (g + 1) * 512],
                                start=(g == 0), stop=(g == 2),
                            )
                    th2 = sb3.tile([128, 2, 512], BF16, tag="th2")
                    nc.scalar.activation(
                        th2.rearrange("p a b -> p (a b)"),
                        sacc2.rearrange("p a b -> p (a b)"),
                        mybir.ActivationFunctionType.Tanh)
                    for h2 in range(2):
                        junk = sb3.tile([128, 512], BF16, tag="junk")
                        nc.vector.scalar_tensor_tensor(
                            junk[:], th2[:, h2, :], 1.0, wrep_sb[:],
                            op0=mybir.AluOpType.mult,
                            op1=mybir.AluOpType.mult,
                            accum_out=scores_sb[:, 2 * p + h2:2 * p + h2 + 1],
                        )

                table_hm = big.tile([64, 512], dt, tag="table_hm")
                n_sib_pair = n_sib_tile // 2
                n_arc_pair = n_arc_tile // 2
                # arc tiles are bucketed by (table half, m 64-chunk); half-0
                # tiles form an even-length prefix and only need table rows
                # 0:32, so they can interleave while rows 32:63 compute.
                n_h0_pair = sum(1 for b in arc_buckets if b < 8) // 2

                with tc.tile_pool(name="ps_tblw", bufs=2, space="PSUM") as ps_w, \
                     tc.tile_pool(name="ps_arc", bufs=2, space="PSUM") as ps_a:

                    def emit_row(hl):
                        tmp = sb3.tile([128, 4, 512], BF16, tag="tmp")
                        nc.vector.tensor_add(
                            tmp[:], mT[:],
                            dwin_sb[:, :, 63 - hl:63 - hl + 512])
                        tht = sb3.tile([128, 4, 512], BF16, tag="tht")
                        for jc in range(4):
                            nc.scalar.activation(
                                tht[:, jc, :], tmp[:, jc, :],
                                mybir.ActivationFunctionType.Tanh,
                                bias=hwin[:, jc, hl:hl + 1])
                        wps = ps_w.tile([128, 512], dt, tag="wps")
                        for jc in range(4):
                            nc.tensor.matmul(
                                wps[:], lhsT=wrepT_sb[:, jc, :],
                                rhs=tht[:, jc, :],
                                start=(jc == 0), stop=(jc == 3),
                            )
                        trow = sb3.tile([128, 512], dt, tag="trow")
                        nc.vector.tensor_copy(trow[:], wps[:])
                        nc.sync.dma_start(tdram[hl:hl + 1, :], trow[0:1, :])

                    def arc_pair(pa):
                        # arc tiles u=2pa, 2pa+1 with host-uploaded h one-hots
                        u = 2 * pa
                        ohh = sb3.tile([64, 2, 128], dt, tag="ohh")
                        nc.sync.dma_start(
                            ohh.rearrange("p a b -> p (a b)"),
                            arc_oh_in[pa, :, :],
                        )
                        comb = ps_a.tile([128, 2, 64], dt, tag="comb")
                        for h2 in range(2):
                            bucket = arc_buckets[u + h2]
                            mq = bucket % 8
                            nc.tensor.matmul(
                                comb[:, h2, :], lhsT=ohh[:, h2, :],
                                rhs=table_hm[0:64, mq * 64:(mq + 1) * 64],
                                start=True, stop=True,
                            )
                        for h2 in range(2):
                            junk2 = sb3.tile([128, 64], BF16, tag="junk2")
                            nc.vector.scalar_tensor_tensor(
                                junk2[:], iota_row[:, 0:64],
                                arcm_sb[:, u + h2:u + h2 + 1],
                                comb[:, h2, :],
                                op0=mybir.AluOpType.is_equal,
                                op1=mybir.AluOpType.mult,
                                accum_out=scores_sb[:, n_sib_tile + u + h2:
                                                    n_sib_tile + u + h2 + 1],
                            )

                    cur = {"sib": 0, "arc": 0}

                    def sibs(n):
                        k = min(n, n_sib_pair - cur["sib"])
                        for _ in range(k):
                            sib_pair(cur["sib"])
                            cur["sib"] += 1

                    def arcs(n):
                        k = min(n, n_arc_pair - cur["arc"])
                        for _ in range(k):
                            arc_pair(cur["arc"])
                            cur["arc"] += 1

                    # phase A: table rows with a few sib pairs for PE overlap
                    for hl in range(64):
                        emit_row(hl)
                        if hl % 3 != 0:
                            sibs(1)
                    nc.sync.dma_start(table_hm[:], tdram[:])
                    # phase B: remaining sib pairs with arc runs between (runs
                    # amortize the PE mode switch between the 128-row sib
                    # gathers and the 64-row arc gathers)
                    while cur["sib"] < n_sib_pair or cur["arc"] < n_arc_pair:
                        sibs(1)
                        arcs(3)

                nc.sync.dma_start(scores_out[:], scores_sb[:])
    return nc


_CACHE = {}


def _get_program(sib_combos, arc_buckets):
    key = (tuple(sib_combos), tuple(arc_buckets))
    if _CACHE.get("key") != key:
        nc = bass.Bass()
        _build(nc, sib_combos, arc_buckets)
        _legalize_waits(nc)
        _CACHE["nc"] = nc
        _CACHE["key"] = key
    return _CACHE["nc"]


def _host_prepare(inputs):
    import jax.numpy as jnp
    import ml_dtypes
    _BF = ml_dtypes.bfloat16

    def bf(x):
        return np.asarray(jnp.asarray(np.asarray(x, np.float32), jnp.bfloat16))

    f32 = np.float32
    words = np.asarray(inputs["words"]).astype(np.int64)
    tags = np.asarray(inputs["tags"]).astype(np.int64)
    word_emb = np.asarray(inputs["word_emb"], f32)
    tag_emb = np.asarray(inputs["tag_emb"], f32)
    emb = np.concatenate([word_emb[words], tag_emb[tags]], axis=-1)  # [512, 364]
    emb_aug = np.concatenate([emb, np.ones((S, 1), f32)], axis=1)    # [512, 365]

    def packT(x, rows):  # -> [rows(pad), ...] = x.T zero-padded
        out = np.zeros((rows, x.shape[0]), f32)
        out[: x.shape[1]] = x.T
        return out

    # permuted position order used by all feature-major [feat, pos] tiles:
    # column p' holds position IPERM[p'];  PERM[pos] = (pos%8)*64 + pos//8
    PERM = (np.arange(S) % 8) * 64 + np.arange(S) // 8
    IPERM = np.argsort(PERM)

    embT_f = bf(packT(emb_aug, 384)[:, IPERM]
                .reshape(3, 128, 512).transpose(1, 0, 2))
    embT_b = bf(packT(emb_aug[::-1], 384)[:, IPERM]
                .reshape(3, 128, 512).transpose(1, 0, 2))

    def wih_pack(Wih, bih, bhh, kdim, rows):
        out = np.zeros((rows, 2, 4 * H), f32)
        for d in range(2):
            out[:kdim, d] = np.asarray(Wih[d], f32).T[:, GPERM]
            out[kdim, d] = (np.asarray(bih[d], f32) + np.asarray(bhh[d], f32))[GPERM]
        return out

    wih0T = bf(wih_pack(inputs["Wih0"], inputs["bih0"], inputs["bhh0"], 364, 384))
    wih1T = bf(wih_pack(inputs["Wih1"], inputs["bih1"], inputs["bhh1"], 1024, 1152))

    def whh_pack(Whh):
        out = np.zeros((128, 4, 2, 4 * H), f32)
        for d in range(2):
            wt = np.asarray(Whh[d], f32).T[:, GPERM]  # [512 k, 2048 g]
            out[:, :, d, :] = wt.reshape(4, 128, 4 * H).transpose(1, 0, 2)
        return out

    whh0T = bf(whh_pack(inputs["Whh0"]))
    whh1T = bf(whh_pack(inputs["Whh1"]))

    projs = [inputs["head_W"], inputs["mod_W"], inputs["sib_head_W"],
             inputs["sib_mod_W"], inputs["sib_sib_W"]]
    projT = np.zeros((1152, 5 * H), f32)
    for i, W in enumerate(projs):
        projT[:1024, i * H:(i + 1) * H] = np.asarray(W, f32).T
    projT = bf(projT)

    w = np.asarray(inputs["arc_w"], f32).reshape(512)
    wrep = bf(np.broadcast_to(w, (128, 512)))
    wrepT = bf(w.reshape(4, 128).T.reshape(128, 4, 1).repeat(128, axis=2))

    # Dfull[off] = D[distidx(off - 511)], off in [0, 1022]
    D = (np.asarray(inputs["dist_emb"], f32) @ np.asarray(inputs["dist_W"], f32).T
         + np.asarray(inputs["dist_b"], f32))
    offs = np.arange(-511, 512)
    bi = np.searchsorted(BINS, np.abs(offs), side="right") - 1
    Dfull = D[np.where(offs > 0, bi, bi + NB)]          # [1023, H]
    DfullT = Dfull.T                                     # [H, 1023]

    iotar = np.tile(np.arange(128, dtype=f32), (128, 1))
    mask = np.zeros((128, 2), f32)
    for mi, s in enumerate((K_WARM - 9, K_WARM - 1)):
        c = np.arange(64)
        v = ((8 * c + s) > (K_WARM - 1)).astype(f32)
        mask[0:64, mi] = v
        mask[64:128, mi] = v

    base = {
        "embT_f": embT_f, "embT_b": embT_b,
        "wih0T": wih0T, "whh0T": whh0T, "wih1T": wih1T, "whh1T": whh1T,
        "projT": projT, "wrep_in": wrep, "wrepT_in": wrepT,
        "iotar_in": iotar, "mask_in": mask,
    }

    ah = np.asarray(inputs["arc_head"]).astype(np.int64)
    am = np.asarray(inputs["arc_mod"]).astype(np.int64)
    sh_i = np.asarray(inputs["sib_head"]).astype(np.int64)
    sm_i = np.asarray(inputs["sib_mod"]).astype(np.int64)
    ss_i = np.asarray(inputs["sib_sib"]).astype(np.int64)

    # ---- global tile layouts (uniform across cores; program depends on them)
    # sibs are sharded BY COMBO: combo c's parts split evenly over cores, each
    # core gets ceil(cnt_g[c]/(128*NC)) tiles for combo c.
    sh_p, sm_p, ss_p = PERM[sh_i], PERM[sm_i], PERM[ss_i]
    combo_g = (sh_p // 128) * 16 + (sm_p // 128) * 4 + (ss_p // 128)
    cnt_g = np.bincount(combo_g, minlength=64)
    sib_tpc = -(-cnt_g // (128 * NC))              # tiles per combo per core
    sib_combos = [c for c in range(64) for _ in range(sib_tpc[c])]
    if len(sib_combos) % 2:
        sib_combos.append(int(np.argmax(sib_tpc == 0)) if (sib_tpc == 0).any()
                          else 0)
        sib_pad = 1
    else:
        sib_pad = 0
    n_sib_tile = len(sib_combos)
    sib_tile_off = np.zeros(65, np.int64)          # first tile of each combo
    np.cumsum(sib_tpc, out=sib_tile_off[1:65])
    sib_ids_by_combo = [np.nonzero(combo_g == c)[0] for c in range(64)]

    # arcs stay h-sharded (core owns a 64-row table slice); buckets are
    # (table-half of local h) x (m 64-chunk); bucket tile counts take the
    # max over cores so the layout is core-uniform.  The half-0 prefix is
    # kept even so arc PAIRS never straddle the half boundary.
    core_of = ah // 64
    arc_ids_core = [np.nonzero(core_of == core)[0] for core in range(NC)]
    cnt_ab = np.zeros((NC, 16), np.int64)
    for core in range(NC):
        ids = arc_ids_core[core]
        ab = ((ah[ids] - 64 * core) // 32) * 8 + am[ids] // 64
        cnt_ab[core] = np.bincount(ab, minlength=16)
    arc_tpb = (-(-cnt_ab.max(axis=0) // 128)).astype(np.int64)
    if arc_tpb[:8].sum() % 2:
        arc_tpb[7] += 1
    if arc_tpb.sum() % 2:
        arc_tpb[15] += 1
    arc_buckets = [b for b in range(16) for _ in range(arc_tpb[b])]
    n_arc_tile = len(arc_buckets)
    arc_tile_off = np.zeros(17, np.int64)
    np.cumsum(arc_tpb, out=arc_tile_off[1:17])

    in_maps = []
    meta = {"arc_slots": [], "sib_slots": [],
            "sib_combos": sib_combos, "arc_buckets": arc_buckets}
    for core in range(NC):
        m = dict(base)
        # per-core D window (transposed): cols [448-64c, 1023-64c), zero-pad to 576
        win = np.zeros((512, 576), f32)
        win[:, :575] = DfullT[:, 448 - 64 * core:1023 - 64 * core]
        m["dwin_in"] = bf(win.reshape(4, 128, 576).transpose(1, 0, 2))
        hsel = np.zeros((512, 64), f32)
        hsel[PERM[64 * core + np.arange(64)], np.arange(64)] = 1.0
        m["hsel_in"] = bf(hsel.reshape(4, 128, 64).transpose(1, 0, 2))

        # arcs owned by this core (h in [64c, 64c+64))
        ids = arc_ids_core[core]
        ab = ((ah[ids] - 64 * core) // 32) * 8 + am[ids] // 64
        arc_slot = np.full(n_arc_tile * 128, -1, np.int64)
        order_a = np.argsort(ab, kind="stable")
        pos = 0
        for b in range(16):
            n = cnt_ab[core][b]
            s0 = arc_tile_off[b] * 128
            arc_slot[s0:s0 + n] = ids[order_a[pos:pos + n]]
            pos += n
        arc_rows = np.zeros((n_arc_tile, 128), np.int64)
        arc_mcol = np.zeros((128, n_arc_tile), f32)
        for t in range(n_arc_tile):
            sel = arc_slot[t * 128:(t + 1) * 128]
            valid = sel >= 0
            b = arc_buckets[t]
            arc_rows[t, valid] = ah[sel[valid]] - 64 * core
            arc_mcol[valid, t] = am[sel[valid]] - 64 * (b % 8)
            # invalid slots must still one-hot a row inside this bucket's half
            arc_rows[t, ~valid] = 32 * (b // 8)
        assert arc_rows.min() >= 0 and arc_rows.max() < 64
        aoh = np.zeros((n_arc_tile // 2, 64, 256), f32)
        avals = arc_rows.reshape(n_arc_tile // 2, 2, 128)
        acols = (np.arange(2)[:, None] * 128 + np.arange(128)[None, :])
        aoh[np.arange(n_arc_tile // 2)[:, None, None], avals, acols[None]] = 1
        m["arc_oh_in"] = aoh
        m["arcm_in"] = arc_mcol
        meta["arc_slots"].append(arc_slot)

        # sibs: this core's share of each combo, packed into the combo's tiles
        sib_slot = np.full(n_sib_tile * 128, -1, np.int64)
        for c in range(64):
            gids = sib_ids_by_combo[c]
            n = len(gids)
            base_n, rem = divmod(n, NC)
            lo = core * base_n + min(core, rem)
            hi = lo + base_n + (1 if core < rem else 0)
            part = gids[lo:hi]
            s0 = sib_tile_off[c] * 128
            assert len(part) <= sib_tpc[c] * 128
            sib_slot[s0:s0 + len(part)] = part
        idx_rows = np.zeros((n_sib_tile, 3, 128), np.int64)
        for t in range(n_sib_tile):
            c = sib_combos[t]
            hc, mc_, sc_ = c // 16, (c // 4) % 4, c % 4
            sel = sib_slot[t * 128:(t + 1) * 128]
            valid = sel >= 0
            sv = np.where(valid, sel, 0)
            idx_rows[t, 0] = np.where(valid, sh_p[sv] - 128 * hc, 0)
            idx_rows[t, 1] = np.where(valid, sm_p[sv] - 128 * mc_, 0)
            idx_rows[t, 2] = np.where(valid, ss_p[sv] - 128 * sc_, 0)
        assert idx_rows.max() < 128 and idx_rows.min() >= 0
        soh = np.zeros((n_sib_tile // 2, 128, 768), _BF)
        svals = idx_rows.reshape(n_sib_tile // 2, 2, 3, 128)
        scols = (np.arange(2)[:, None, None] * 384
                 + np.arange(3)[None, :, None] * 128
                 + np.arange(128)[None, None, :])
        soh[np.arange(n_sib_tile // 2)[:, None, None, None], svals,
            scols[None]] = 1
        # padding tile shares a real combo's one-hot slot: zero it out
        if sib_pad:
            soh[-1, :, 384:768] = 0
        m["sib_oh_in"] = soh
        meta["sib_slots"].append(sib_slot)
        in_maps.append(m)
    return in_maps, meta


LAST_EXEC_NS = None


def kernel(**inputs):
    global LAST_EXEC_NS
    _install_ntff_hook()
    from concourse.bass_utils import run_bass_kernel_spmd

    in_maps, meta = _host_prepare(inputs)
    nc = _get_program(meta["sib_combos"], meta["arc_buckets"])
    import os

    trace = os.environ.get("KERNEL_TRACE", "0") == "1"
    res = run_bass_kernel_spmd(nc, in_maps, list(range(NC)), trace=trace)
    LAST_EXEC_NS = res.exec_time_ns
    _CACHE["res"] = res
    n_sib_tile = len(meta["sib_combos"])
    arc_scores = np.zeros(A, np.float32)
    sib_scores = np.zeros(ASIB, np.float32)
    for core in range(NC):
        sc = np.asarray(res.results[core]["scores_out"])  # [128, n_tile]
        sib_flat = sc[:, :n_sib_tile].T.reshape(-1)
        sib_slot = meta["sib_slots"][core]                # global sib ids
        valid = sib_slot >= 0
        sib_scores[sib_slot[valid]] = sib_flat[valid]

        arc_flat = sc[:, n_sib_tile:].T.reshape(-1)
        arc_slot = meta["arc_slots"][core]                # global arc ids
        valid = arc_slot >= 0
        arc_scores[arc_slot[valid]] = arc_flat[valid]
    return np.concatenate([arc_scores, sib_scores])



# revision 89
# speedup vs baseline: 1.0846x; 1.0846x over previous
"""Trainium2 Bass kernel for nn_DependencyNeuralModel (dependency parser scorer).

v2 design (8 NeuronCores, SPMD):
  Encoder: 2-layer BiLSTM over S=512, replicated on every core, chunk-parallel
    (64 chunks x 2 dirs advance lock-step as 128 rows through the PE).
    K_WARM=16 warmup steps; gate order repacked to [i,f,o,g] so the i/f
    half of the recurrent GEMM can overlap the o/g half's activations.
  Arc scores: score(h,m) depends only on the (h,m) pair (dist is a function
    of m-h), so each core builds the 64-row slice of the full SxS score
    table it owns (h sharded), in a transposed layout where the dist term
    is a contiguous slice of a host-built [H, 1023] offset table and the
    head term is a per-partition activation-fused bias.  The per-arc gather
    is then a single GPSIMD ap_gather of scalar (pair) entries from the
    partition-replicated table; host picks the parity lane and unsorts.
  Sib scores: part-sharded; host sorts each core's 16384 parts by the
    (head,mod,sib) 128-chunk combo (64 combos x 3 static tiles), so each
    128-part tile needs only 3 one-hot gather matmuls instead of 12.
Host does only index/layout preparation and final unshard.
"""
import sys
import types

import numpy as np

sys.path.insert(0, "/opt/trn_rl_repo")

import concourse.bass as bass
import concourse.mybir as mybir
from concourse.tile import TileContext
from concourse.masks import make_identity

S = 512
H = 512
A = 262144
ASIB = 131072
NB = 17
L = 8
K_WARM = 14
NSTEP = K_WARM + L  # 24
NC = 8
F32 = mybir.dt.float32
BF16 = mybir.dt.bfloat16
U16 = mybir.dt.uint16
BINS = np.array(list(range(10)) + list(range(10, 40, 5)) + [40], dtype=np.int64)

GPERM = np.r_[0:1024, 1536:2048, 1024:1536]  # gate reorder i,f,g,o -> i,f,o,g


def _install_ntff_hook():
    if "antenv.axon_hooks" in sys.modules:
        return
    mod = types.ModuleType("antenv.axon_hooks")
    state = {"hook": None, "tried": False}

    def set_axon_ntff_profile_hook(hook):
        state["hook"] = hook

    def get_axon_ntff_profile_hook():
        if state["hook"] is None and not state["tried"]:
            state["tried"] = True
            try:
                from trn_agent_boot.trn_boot import _ntff_profile_via_ctypes

                state["hook"] = _ntff_profile_via_ctypes("/opt/axon/libaxon_pjrt.so")
            except Exception:
                state["hook"] = None
        return state["hook"]

    mod.set_axon_ntff_profile_hook = set_axon_ntff_profile_hook
    mod.get_axon_ntff_profile_hook = get_axon_ntff_profile_hook
    import antenv

    antenv.axon_hooks = mod
    sys.modules["antenv.axon_hooks"] = mod


def _legalize_waits(nc):
    """This walrus accepts at most one semaphore wait per instruction;
    split extra waits onto same-engine NOPs placed just before."""
    ctr = [0]
    for f in nc.m.functions:
        for blk in f.blocks:
            out = []
            dirty = False
            for ins in blk.instructions:
                si = ins.sync_info
                if si is not None and si.on_wait and len(si.on_wait) > 1:
                    waits = list(si.on_wait)
                    for w in waits[:-1]:
                        ctr[0] += 1
                        nop = mybir.InstNoOp(name=f"waitfix-{ctr[0]}")
                        nop.engine = ins.engine
                        nop.sync_info = mybir.SyncInfo(on_wait=[w], on_update=[])
                        out.append(nop)
                    ins.sync_info = mybir.SyncInfo(
                        on_wait=[waits[-1]],
                        on_update=list(si.on_update) if si.on_update else [],
                    )
                    dirty = True
                out.append(ins)
            if dirty:
                blk.instructions = out
    return nc


def _lstm_layer(nc, tc, ident, mask_sb, whhT_dram, wx_dram, dstT, dstTrev,
                whh_pre=None):
    """One BiLSTM layer, chunk-parallel.  B=128 rows: partitions 0:64 are
    dir0 chunks, 64:128 dir1 chunks.  Gate columns are [i,f,o,g]; the
    i/f half of the recurrent GEMM is emitted first so its activations
    overlap the o/g half's matmuls.  Output states are written directly
    into the feature-major [128, 9, 512] tiles dstT (and optionally the
    position-reversed dstTrev) from the per-step transposed state."""
    import contextlib

    with contextlib.ExitStack() as ctx:
        sb = ctx.enter_context(tc.tile_pool(name="lstm_sb", bufs=4))
        cold = ctx.enter_context(tc.tile_pool(name="lstm_cold", bufs=1))
        st = ctx.enter_context(tc.tile_pool(name="lstm_state", bufs=1))
        ps1 = ctx.enter_context(tc.tile_pool(name="lstm_ps1", bufs=1, space="PSUM"))
        ps2 = ctx.enter_context(tc.tile_pool(name="lstm_ps2", bufs=1, space="PSUM"))
        pst = ctx.enter_context(tc.tile_pool(name="lstm_pst", bufs=2, space="PSUM"))

        if whh_pre is not None:
            whh_sb = whh_pre
        else:
            whh_sb = st.tile([128, 4, 2, 2048], BF16)
            nc.sync.dma_start(
                whh_sb.rearrange("p a b c -> p (a b c)"),
                whhT_dram.rearrange("p a b c -> p (a b c)"),
            )
        h_t = st.tile([128, 4, 128], BF16)  # h transposed: [k-part, kc, b]
        c_st = st.tile([128, 512], BF16)    # [b, k]
        nc.vector.memset(h_t.rearrange("p a b -> p (a b)"), 0.0)
        nc.vector.memset(c_st[:], 0.0)

        def fetch_wx(s):
            wx = sb.tile([128, 2048], BF16, tag="wx")
            for d in range(2):
                nc.sync.dma_start(
                    wx[d * 64:(d + 1) * 64, :], wx_dram[d, s:s + 505:8, :]
                )
            return wx

        def emit_ident(wx):
            """identity-injection matmuls; emitted during the previous
            step's tail so the PE fills otherwise-idle cycles."""
            g01 = ps1.tile([128, 1024], F32, tag="g01")
            g23 = ps2.tile([128, 1024], F32, tag="g23")
            for half, gps in ((0, g01), (1, g23)):
                for ng in range(2):
                    col = (half * 2 + ng) * 512
                    for d in range(2):
                        bs = slice(d * 64, (d + 1) * 64)
                        nc.tensor.matmul(
                            gps[bs, ng * 512:(ng + 1) * 512],
                            lhsT=ident[:, bs],
                            rhs=wx[:, col:col + 512],
                            start=True, stop=False,
                        )
            return g01, g23

        wxq = {0: fetch_wx(0), 1: fetch_wx(1), 2: fetch_wx(2)}
        nxt = emit_ident(wxq.pop(0))
        for s in range(NSTEP):
            # recurrent half of the gates GEMM.  d0 writes psum rows 0:64
            # (array cols 0-63), d1 rows 64:128 (cols 64-127); adjacent
            # d0/d1 matmuls run concurrently as (128,64) column tiles.
            g01, g23 = nxt
            for half, gps in ((0, g01), (1, g23)):
                for ng in range(2):
                    col = (half * 2 + ng) * 512
                    for kc in range(4):
                        for d in range(2):
                            bs = slice(d * 64, (d + 1) * 64)
                            nc.tensor.matmul(
                                gps[bs, ng * 512:(ng + 1) * 512],
                                lhsT=h_t[:, kc, bs],
                                rhs=whh_sb[:, kc, d, col:col + 512],
                                start=False,
                                stop=(kc == 3),
                            )
            sig_if = cold.tile([128, 1024], BF16, tag="sif")
            nc.scalar.activation(sig_if[:], g01[:],
                                 mybir.ActivationFunctionType.Sigmoid)
            tanh_g = cold.tile([128, 512], BF16, tag="tg")
            nc.scalar.activation(tanh_g[:], g23[:, 512:1024],
                                 mybir.ActivationFunctionType.Tanh)
            sig_o = cold.tile([128, 512], BF16, tag="so")
            nc.scalar.activation(sig_o[:], g23[:, 0:512],
                                 mybir.ActivationFunctionType.Sigmoid)
            t1 = cold.tile([128, 512], BF16, tag="t1")
            nc.vector.tensor_mul(t1[:], sig_if[:, 512:1024], c_st[:])
            t2 = cold.tile([128, 512], BF16, tag="t2")
            nc.vector.tensor_mul(t2[:], sig_if[:, 0:512], tanh_g[:])
            nc.vector.tensor_add(c_st[:], t1[:], t2[:])
            tch = cold.tile([128, 512], BF16, tag="tch")
            nc.scalar.activation(tch[:], c_st[:], mybir.ActivationFunctionType.Tanh)
            h_new = cold.tile([128, 512], BF16, tag="h")
            nc.vector.tensor_mul(h_new[:], sig_o[:], tch[:])
            if s in (K_WARM - 9, K_WARM - 1):
                mi = {K_WARM - 9: 0, K_WARM - 1: 1}[s]
                nc.vector.tensor_scalar_mul(h_new[:], h_new[:], mask_sb[:, mi:mi + 1])
                nc.vector.tensor_scalar_mul(c_st[:], c_st[:], mask_sb[:, mi:mi + 1])
            if s + 3 < NSTEP:
                wxq[s + 3] = fetch_wx(s + 3)
            if s + 1 < NSTEP:
                # next step's identity matmuls go into the PE queue BEFORE
                # this step's transposes: they have no h dependency and run
                # while the tail above executes.
                nxt = emit_ident(wxq.pop(s + 1))
            tp = pst.tile([128, 4, 128], BF16, tag="tr_ps")
            for kc in range(4):
                nc.tensor.transpose(tp[:, kc, :],
                                    h_new[:, kc * 128:(kc + 1) * 128], ident[:])
            nc.vector.tensor_copy(h_t.rearrange("p a b -> p (a b)"),
                                  tp.rearrange("p a b -> p (a b)"))
            if s >= K_WARM:
                o = s - K_WARM
                # scatter this step's transposed states into the
                # feature-major output tiles, which use the PERMUTED
                # position order p' = (pos%8)*64 + pos//8 so every write
                # is a contiguous 64-column block.  dir0 chunk c is
                # position 8c+o -> block o; dir1 (backward) chunk c is
                # position 511-(8c+o) -> block 7-o, chunk axis reversed.
                nc.vector.tensor_copy(dstT[:, 0:4, o * 64:(o + 1) * 64],
                                      tp[:, :, 0:64])
                nc.vector.tensor_copy(
                    dstT[:, 4:8, (7 - o) * 64:(8 - o) * 64],
                    tp[:, :, 127:63:-1])
                if dstTrev is not None:
                    nc.vector.tensor_copy(
                        dstTrev[:, 0:4, (7 - o) * 64:(8 - o) * 64],
                        tp[:, :, 63::-1])
                    nc.vector.tensor_copy(
                        dstTrev[:, 4:8, o * 64:(o + 1) * 64],
                        tp[:, :, 64:128])


def _input_gemm(nc, tc, lhsT_tiles, wihT_dram, wx_dram, nk, klast, pre=None):
    """WX[d] = lhsT_d.T @ wihT[d] -> wx_dram[d, K_WARM:K_WARM+512, :].
    lhsT_tiles: per-dir tile [128, nk, 512] in SBUF ([feat-part, chunk, pos]).
    nk chunks; last chunk has klast valid rows.  If pre is given it is an
    SBUF-resident [128, nk, 2, 2048] copy of the weights (prefetched long
    before, so this GEMM issues no DMA reads at all)."""
    import contextlib

    with contextlib.ExitStack() as ctx:
        sb = ctx.enter_context(tc.tile_pool(name="ig_sb", bufs=6))
        ps = ctx.enter_context(tc.tile_pool(name="ig_ps", bufs=2, space="PSUM"))
        for d in range(2):
            lhsT = lhsT_tiles[d]
            for ngc in range(4):
                acc4 = ps.tile([128, 4, 512], F32, tag="acc4")
                for kc in range(nk):
                    kk = 128 if kc < nk - 1 else klast
                    if pre is not None:
                        rhs_ap = pre[:kk, kc, d, ngc * 512:(ngc + 1) * 512]
                    else:
                        rhs = sb.tile([128, 512], wihT_dram.dtype, tag="rhs")
                        nc.sync.dma_start(
                            rhs[:kk, :],
                            wihT_dram[kc * 128:kc * 128 + kk, d,
                                      ngc * 512:(ngc + 1) * 512],
                        )
                        rhs_ap = rhs[:kk, :]
                    for mc in range(4):
                        nc.tensor.matmul(
                            acc4[:, mc, :],
                            lhsT=lhsT[:kk, kc, mc * 128:(mc + 1) * 128],
                            rhs=rhs_ap,
                            start=(kc == 0),
                            stop=(kc == nk - 1),
                        )
                osb = sb.tile([128, 4, 512], BF16, tag="osb")
                nc.scalar.activation(
                    osb.rearrange("p a b -> p (a b)"),
                    acc4.rearrange("p a b -> p (a b)"),
                    mybir.ActivationFunctionType.Copy)
                # lhsT columns are in permuted position order
                # p' = o*64 + c (o = 2*mc + a); scatter rows back to the
                # natural sliding-window rows 8c + o of wx_dram.
                for mc in range(4):
                    for a in range(2):
                        r0 = K_WARM + 2 * mc + a
                        nc.sync.dma_start(
                            wx_dram[d, r0:r0 + 505:8,
                                    ngc * 512:(ngc + 1) * 512],
                            osb[a * 64:(a + 1) * 64, mc, :],
                        )


def _build(nc, sib_combos, arc_buckets):
    dt = F32
    n_sib_tile = len(sib_combos)       # even
    n_arc_tile = len(arc_buckets)      # even
    n_tile = n_sib_tile + n_arc_tile
    embT_f = nc.dram_tensor("embT_f", [128, 3, 512], BF16, kind="ExternalInput")
    embT_b = nc.dram_tensor("embT_b", [128, 3, 512], BF16, kind="ExternalInput")
    wih0T = nc.dram_tensor("wih0T", [384, 2, 2048], BF16, kind="ExternalInput")
    whh0T = nc.dram_tensor("whh0T", [128, 4, 2, 2048], BF16, kind="ExternalInput")
    wih1T = nc.dram_tensor("wih1T", [1152, 2, 2048], BF16, kind="ExternalInput")
    whh1T = nc.dram_tensor("whh1T", [128, 4, 2, 2048], BF16, kind="ExternalInput")
    projT = nc.dram_tensor("projT", [1152, 2560], BF16, kind="ExternalInput")
    dwin_in = nc.dram_tensor("dwin_in", [128, 4, 576], BF16, kind="ExternalInput")
    hsel_in = nc.dram_tensor("hsel_in", [128, 4, 64], BF16, kind="ExternalInput")
    wrep_in = nc.dram_tensor("wrep_in", [128, 512], BF16, kind="ExternalInput")
    wrepT_in = nc.dram_tensor("wrepT_in", [128, 4, 128], BF16, kind="ExternalInput")
    sib_oh_in = nc.dram_tensor("sib_oh_in", [n_sib_tile // 2, 128, 768], BF16,
                               kind="ExternalInput")
    arc_oh_in = nc.dram_tensor("arc_oh_in", [n_arc_tile // 2, 64, 256], dt,
                               kind="ExternalInput")
    arcm_in = nc.dram_tensor("arcm_in", [128, n_arc_tile], dt,
                             kind="ExternalInput")
    iotar_in = nc.dram_tensor("iotar_in", [128, 128], dt, kind="ExternalInput")
    mask_in = nc.dram_tensor("mask_in", [128, 2], dt, kind="ExternalInput")
    scores_out = nc.dram_tensor("scores_out", [128, n_tile], dt,
                                kind="ExternalOutput")

    wx0 = nc.dram_tensor("wx0", [2, 544, 2048], BF16)
    tdram = nc.dram_tensor("tdram", [64, 512], F32)
    wx1 = nc.dram_tensor("wx1", [2, 544, 2048], BF16)
    tdram = nc.dram_tensor("tdram", [64, 512], F32)

    import contextlib

    with TileContext(nc) as tc:
        with contextlib.ExitStack() as ctx:
            const = ctx.enter_context(tc.tile_pool(name="const", bufs=1))
            big = ctx.enter_context(tc.tile_pool(name="big", bufs=1))

            ident = const.tile([128, 128], BF16)
            make_identity(nc, ident[:])
            mask_sb = const.tile([128, 2], dt)
            nc.sync.dma_start(mask_sb[:], mask_in[:])
            one_row = const.tile([1, 512], BF16)
            nc.vector.memset(one_row[:], 1.0)
            wrep_sb = const.tile([128, 512], BF16)
            wrepT_sb = const.tile([128, 4, 128], BF16)
            dwin_sb = const.tile([128, 4, 576], BF16)
            hsel_sb = const.tile([128, 4, 64], BF16)
            iota_row = const.tile([128, 128], dt)
            arcm_sb = const.tile([128, n_arc_tile], dt)

            # zero-pad warmup rows of WX buffers
            with tc.tile_pool(name="zp", bufs=1) as zp:
                zrow = zp.tile([64, 2048], BF16)
                nc.vector.memset(zrow[:], 0.0)
                for wxd in (wx0, wx1):
                    for d in range(2):
                        nc.sync.dma_start(wxd[d, 0:K_WARM, :], zrow[0:K_WARM, :])
                        nc.sync.dma_start(wxd[d, K_WARM + 512:544, :],
                                          zrow[0:32 - K_WARM, :])

            # ---- layer 0 (streams states into x1T / x1Trev) ----
            x1T = big.tile([128, 9, 512], BF16, tag="x1T")
            x1Trev = big.tile([128, 9, 512], BF16, tag="x1Trev")
            for dst in (x1T, x1Trev):
                nc.vector.memset(dst[:, 8, :], 0.0)
                nc.vector.tensor_copy(dst[0:1, 8, :], one_row[:])

            with tc.tile_pool(name="w1pre", bufs=1) as w1p:
                # ---- WX0 + weight prefetch ----
                # the sync queue stalls at WX0's first sem-gated output
                # write, so any DMA issued after WX0 starts ~30us late.
                # Issue embeddings first, then layer-0 recurrent weights,
                # then the wih1T prefetch -- all BEFORE the WX0 GEMM body.
                with tc.tile_pool(name="emb_sb", bufs=1) as emb_pool:
                    ef = emb_pool.tile([128, 3, 512], BF16)
                    nc.sync.dma_start(ef.rearrange("p a b -> p (a b)"),
                                      embT_f.rearrange("p a b -> p (a b)"))
                    eb = emb_pool.tile([128, 3, 512], BF16)
                    nc.sync.dma_start(eb.rearrange("p a b -> p (a b)"),
                                      embT_b.rearrange("p a b -> p (a b)"))
                    whh_sb = big.tile([128, 4, 2, 2048], BF16, tag="whh")
                    nc.sync.dma_start(
                        whh_sb.rearrange("p a b c -> p (a b c)"),
                        whh0T.rearrange("p a b c -> p (a b c)"))
                    _input_gemm(nc, tc, [ef, eb], wih0T, wx0, 3, 128)
                # wih1T prefetch: queued behind WX0's sync-stalls is fine --
                # it only has to complete before WX1, ~280us later.
                w1sb = w1p.tile([128, 9, 2, 2048], BF16, tag="w1")
                for kc in range(9):
                    for d in range(2):
                        nc.sync.dma_start(
                            w1sb[:, kc, d, :],
                            wih1T[kc * 128:(kc + 1) * 128, d, :])

                _lstm_layer(nc, tc, ident, mask_sb, whh0T, wx0, x1T, x1Trev,
                            whh_pre=whh_sb)

                # reload the shared recurrent-weight tile with layer 1's
                # weights; transfers during WX1 so layer 1 starts instantly
                nc.sync.dma_start(
                    whh_sb.rearrange("p a b c -> p (a b c)"),
                    whh1T.rearrange("p a b c -> p (a b c)"))

                # scoring-only constants load behind the startup-critical DMAs
                nc.sync.dma_start(wrep_sb[:], wrep_in[:])
                nc.sync.dma_start(wrepT_sb.rearrange("p a b -> p (a b)"),
                                  wrepT_in.rearrange("p a b -> p (a b)"))
                nc.sync.dma_start(dwin_sb.rearrange("p a b -> p (a b)"),
                                  dwin_in.rearrange("p a b -> p (a b)"))
                nc.sync.dma_start(hsel_sb.rearrange("p a b -> p (a b)"),
                                  hsel_in.rearrange("p a b -> p (a b)"))
                nc.sync.dma_start(iota_row[:], iotar_in[:])
                nc.sync.dma_start(arcm_sb[:], arcm_in[:])

                # ---- WX1 ----
                _input_gemm(nc, tc, [x1T, x1Trev], wih1T, wx1, 9, 1, pre=w1sb)

            # ---- layer 1 (streams states into stT) ----
            stT = big.tile([128, 9, 512], BF16, tag="x1Trev")  # reuse slot
            nc.vector.memset(stT[:, 8, :], 0.0)
            nc.vector.tensor_copy(stT[0:1, 8, :], one_row[:])
            ppre_pool = ctx.enter_context(tc.tile_pool(name="ppre", bufs=1))
            ppre = ppre_pool.tile([128, 9, 2560], BF16)
            for kc in range(9):
                nc.sync.dma_start(ppre[:, kc, :],
                                  projT[kc * 128:(kc + 1) * 128, :])
            _lstm_layer(nc, tc, ident, mask_sb, whh1T, wx1, stT, None,
                        whh_pre=whh_sb)

            # ---- pos-major projection tables (head + 3 sib; skip mod) ----
            tables_sb = big.tile([128, 4, 2560], BF16, tag="tables")
            with contextlib.ExitStack() as c2:
                sb2 = c2.enter_context(tc.tile_pool(name="tb_sb", bufs=6))
                with tc.tile_pool(name="tb_ps4", bufs=2, space="PSUM") as ps4:
                    for ngc in (0,):
                        acc4 = ps4.tile([128, 4, 512], dt, tag="acc4")
                        for kc in range(9):
                            kk = 128 if kc < 8 else 1
                            for mc in range(4):
                                nc.tensor.matmul(
                                    acc4[:, mc, :],
                                    lhsT=stT[:kk, kc, mc * 128:(mc + 1) * 128],
                                    rhs=ppre[:kk, kc,
                                             ngc * 512:(ngc + 1) * 512],
                                    start=(kc == 0),
                                    stop=(kc == 8),
                                )
                        for mc in range(4):
                            nc.scalar.activation(
                                tables_sb[:, mc, ngc * 512:(ngc + 1) * 512],
                                acc4[:, mc, :],
                                mybir.ActivationFunctionType.Copy)
                ps2 = c2.enter_context(tc.tile_pool(name="tb_ps", bufs=2,
                                                    space="PSUM"))

                # ---- transposed mod table M_T[j, m] ----
                mTp = big.tile([128, 4, 512], BF16, tag="mTp")
                for jc in range(4):
                    acc = ps2.tile([128, 512], dt, tag="acc")
                    for kc in range(8):
                        nc.tensor.matmul(
                            acc[:],
                            lhsT=ppre[:, kc,
                                      512 + jc * 128:512 + (jc + 1) * 128],
                            rhs=stT[:, kc, :],
                            start=(kc == 0), stop=(kc == 7),
                        )
                    nc.scalar.activation(mTp[:, jc, :], acc[:],
                                         mybir.ActivationFunctionType.Copy)
                # un-permute m columns to natural position order (one-time)
                mT = big.tile([128, 4, 512], BF16, tag="mT")
                for o in range(8):
                    nc.vector.tensor_copy(mT[:, :, o::8],
                                          mTp[:, :, o * 64:(o + 1) * 64])

                # ---- H window: hwin[j, hl] = heads[64c+hl, j] ----
                hwin = big.tile([128, 4, 64], dt, tag="hwin")
                for jc in range(4):
                    acc = ps2.tile([128, 64], dt, tag="acch")
                    for kc in range(4):
                        nc.tensor.matmul(
                            acc[:],
                            lhsT=tables_sb[:, kc, jc * 128:(jc + 1) * 128],
                            rhs=hsel_sb[:, kc, :],
                            start=(kc == 0), stop=(kc == 3),
                        )
                    nc.vector.tensor_copy(hwin[:, jc, :], acc[:])

                # sib projection tables AFTER mT/hwin: the scoring rows only
                # need mT+hwin, so their ACT/DVE pipeline starts while these
                # PE-only GEMMs still stream.
                with tc.tile_pool(name="tb_ps5", bufs=1, space="PSUM") as ps5:
                    for ngc in (2, 3, 4):
                        acc4 = ps5.tile([128, 4, 512], dt, tag="acc4b")
                        for kc in range(9):
                            kk = 128 if kc < 8 else 1
                            for mc in range(4):
                                nc.tensor.matmul(
                                    acc4[:, mc, :],
                                    lhsT=stT[:kk, kc, mc * 128:(mc + 1) * 128],
                                    rhs=ppre[:kk, kc,
                                             ngc * 512:(ngc + 1) * 512],
                                    start=(kc == 0),
                                    stop=(kc == 8),
                                )
                        for mc in range(4):
                            nc.scalar.activation(
                                tables_sb[:, mc, ngc * 512:(ngc + 1) * 512],
                                acc4[:, mc, :],
                                mybir.ActivationFunctionType.Copy)

            # ---- scoring ----
            scores_sb = big.tile([128, n_tile], dt, tag="scores")
            with contextlib.ExitStack() as c3:
                sb3 = c3.enter_context(tc.tile_pool(name="sc_sb", bufs=4))
                ps_sacc = c3.enter_context(tc.tile_pool(name="ps_sacc", bufs=2,
                                                        space="PSUM"))

                def sib_pair(p):
                    # sib tiles 2p, 2p+1 with host-uploaded one-hots
                    oh = sb3.tile([128, 2, 3, 128], BF16, tag="oh")
                    nc.sync.dma_start(
                        oh.rearrange("p a b c -> p (a b c)"),
                        sib_oh_in[p, :, :],
                    )
                    sacc2 = ps_sacc.tile([128, 2, 512], dt, tag="sacc2")
                    for h2 in range(2):
                        combo = sib_combos[2 * p + h2]
                        chunks = (combo // 16, (combo // 4) % 4, combo % 4)
                        for g in range(3):
                            nc.tensor.matmul(
                                sacc2[:, h2, :], lhsT=oh[:, h2, g, :],
                                rhs=tables_sb[:, chunks[g],
                                              1024 + g * 512:
                                              1024 + (g + 1) * 512],
                                start=(g == 0), stop=(g == 2),
                            )
                    th2 = sb3.tile([128, 2, 512], BF16, tag="th2")
                    nc.scalar.activation(
                        th2.rearrange("p a b -> p (a b)"),
                        sacc2.rearrange("p a b -> p (a b)"),
                        mybir.ActivationFunctionType.Tanh)
                    for h2 in range(2):
                        junk = sb3.tile([128, 512], BF16, tag="junk")
                        nc.vector.scalar_tensor_tensor(
                            junk[:], th2[:, h2, :], 1.0, wrep_sb[:],
                            op0=mybir.AluOpType.mult,
                            op1=mybir.AluOpType.mult,
                            accum_out=scores_sb[:, 2 * p + h2:2 * p + h2 + 1],
                        )

                table_hm = big.tile([64, 512], dt, tag="table_hm")
                n_sib_pair = n_sib_tile // 2
                n_arc_pair = n_arc_tile // 2
                # arc tiles are bucketed by (table half, m 64-chunk); half-0
                # tiles form an even-length prefix and only need table rows
                # 0:32, so they can interleave while rows 32:63 compute.
                n_h0_pair = sum(1 for b in arc_buckets if b < 8) // 2

                with tc.tile_pool(name="ps_tblw", bufs=2, space="PSUM") as ps_w, \
                     tc.tile_pool(name="ps_arc", bufs=2, space="PSUM") as ps_a:

                    def emit_row(hl):
                        tmp = sb3.tile([128, 4, 512], BF16, tag="tmp")
                        nc.vector.tensor_add(
                            tmp[:], mT[:],
                            dwin_sb[:, :, 63 - hl:63 - hl + 512])
                        tht = sb3.tile([128, 4, 512], BF16, tag="tht")
                        for jc in range(4):
                            nc.scalar.activation(
                                tht[:, jc, :], tmp[:, jc, :],
                                mybir.ActivationFunctionType.Tanh,
                                bias=hwin[:, jc, hl:hl + 1])
                        wps = ps_w.tile([128, 512], dt, tag="wps")
                        for jc in range(4):
                            nc.tensor.matmul(
                                wps[:], lhsT=wrepT_sb[:, jc, :],
                                rhs=tht[:, jc, :],
                                start=(jc == 0), stop=(jc == 3),
                            )
                        trow = sb3.tile([128, 512], dt, tag="trow")
                        nc.vector.tensor_copy(trow[:], wps[:])
                        nc.sync.dma_start(tdram[hl:hl + 1, :], trow[0:1, :])

                    def arc_pair(pa):
                        # arc tiles u=2pa, 2pa+1 with host-uploaded h one-hots
                        u = 2 * pa
                        ohh = sb3.tile([64, 2, 128], dt, tag="ohh")
                        nc.sync.dma_start(
                            ohh.rearrange("p a b -> p (a b)"),
                            arc_oh_in[pa, :, :],
                        )
                        comb = ps_a.tile([128, 2, 64], dt, tag="comb")
                        for h2 in range(2):
                            bucket = arc_buckets[u + h2]
                            mq = bucket % 8
                            nc.tensor.matmul(
                                comb[:, h2, :], lhsT=ohh[:, h2, :],
                                rhs=table_hm[0:64, mq * 64:(mq + 1) * 64],
                                start=True, stop=True,
                            )
                        for h2 in range(2):
                            junk2 = sb3.tile([128, 64], BF16, tag="junk2")
                            nc.vector.scalar_tensor_tensor(
                                junk2[:], iota_row[:, 0:64],
                                arcm_sb[:, u + h2:u + h2 + 1],
                                comb[:, h2, :],
                                op0=mybir.AluOpType.is_equal,
                                op1=mybir.AluOpType.mult,
                                accum_out=scores_sb[:, n_sib_tile + u + h2:
                                                    n_sib_tile + u + h2 + 1],
                            )

                    cur = {"sib": 0, "arc": 0}

                    def sibs(n):
                        k = min(n, n_sib_pair - cur["sib"])
                        for _ in range(k):
                            sib_pair(cur["sib"])
                            cur["sib"] += 1

                    def arcs(n):
                        k = min(n, n_arc_pair - cur["arc"])
                        for _ in range(k):
                            arc_pair(cur["arc"])
                            cur["arc"] += 1

                    # phase A: table rows with a few sib pairs for PE overlap
                    for hl in range(64):
                        emit_row(hl)
                        if hl % 3 == 2:
                            sibs(1)
                    nc.sync.dma_start(table_hm[:], tdram[:])
                    # phase B: remaining sib pairs with arc runs between (runs
                    # amortize the PE mode switch between the 128-row sib
                    # gathers and the 64-row arc gathers)
                    while cur["sib"] < n_sib_pair or cur["arc"] < n_arc_pair:
                        sibs(1)
                        arcs(3)

                nc.sync.dma_start(scores_out[:], scores_sb[:])
    return nc


_CACHE = {}


def _get_program(sib_combos, arc_buckets):
    key = (tuple(sib_combos), tuple(arc_buckets))
    if _CACHE.get("key") != key:
        nc = bass.Bass()
        _build(nc, sib_combos, arc_buckets)
        _legalize_waits(nc)
        _CACHE["nc"] = nc
        _CACHE["key"] = key
    return _CACHE["nc"]


def _host_prepare(inputs):
    import jax.numpy as jnp
    import ml_dtypes
    _BF = ml_dtypes.bfloat16

    def bf(x):
        return np.asarray(jnp.asarray(np.asarray(x, np.float32), jnp.bfloat16))

    f32 = np.float32
    words = np.asarray(inputs["words"]).astype(np.int64)
    tags = np.asarray(inputs["tags"]).astype(np.int64)
    word_emb = np.asarray(inputs["word_emb"], f32)
    tag_emb = np.asarray(inputs["tag_emb"], f32)
    emb = np.concatenate([word_emb[words], tag_emb[tags]], axis=-1)  # [512, 364]
    emb_aug = np.concatenate([emb, np.ones((S, 1), f32)], axis=1)    # [512, 365]

    def packT(x, rows):  # -> [rows(pad), ...] = x.T zero-padded
        out = np.zeros((rows, x.shape[0]), f32)
        out[: x.shape[1]] = x.T
        return out

    # permuted position order used by all feature-major [feat, pos] tiles:
    # column p' holds position IPERM[p'];  PERM[pos] = (pos%8)*64 + pos//8
    PERM = (np.arange(S) % 8) * 64 + np.arange(S) // 8
    IPERM = np.argsort(PERM)

    embT_f = bf(packT(emb_aug, 384)[:, IPERM]
                .reshape(3, 128, 512).transpose(1, 0, 2))
    embT_b = bf(packT(emb_aug[::-1], 384)[:, IPERM]
                .reshape(3, 128, 512).transpose(1, 0, 2))

    def wih_pack(Wih, bih, bhh, kdim, rows):
        out = np.zeros((rows, 2, 4 * H), f32)
        for d in range(2):
            out[:kdim, d] = np.asarray(Wih[d], f32).T[:, GPERM]
            out[kdim, d] = (np.asarray(bih[d], f32) + np.asarray(bhh[d], f32))[GPERM]
        return out

    wih0T = bf(wih_pack(inputs["Wih0"], inputs["bih0"], inputs["bhh0"], 364, 384))
    wih1T = bf(wih_pack(inputs["Wih1"], inputs["bih1"], inputs["bhh1"], 1024, 1152))

    def whh_pack(Whh):
        out = np.zeros((128, 4, 2, 4 * H), f32)
        for d in range(2):
            wt = np.asarray(Whh[d], f32).T[:, GPERM]  # [512 k, 2048 g]
            out[:, :, d, :] = wt.reshape(4, 128, 4 * H).transpose(1, 0, 2)
        return out

    whh0T = bf(whh_pack(inputs["Whh0"]))
    whh1T = bf(whh_pack(inputs["Whh1"]))

    projs = [inputs["head_W"], inputs["mod_W"], inputs["sib_head_W"],
             inputs["sib_mod_W"], inputs["sib_sib_W"]]
    projT = np.zeros((1152, 5 * H), f32)
    for i, W in enumerate(projs):
        projT[:1024, i * H:(i + 1) * H] = np.asarray(W, f32).T
    projT = bf(projT)

    w = np.asarray(inputs["arc_w"], f32).reshape(512)
    wrep = bf(np.broadcast_to(w, (128, 512)))
    wrepT = bf(w.reshape(4, 128).T.reshape(128, 4, 1).repeat(128, axis=2))

    # Dfull[off] = D[distidx(off - 511)], off in [0, 1022]
    D = (np.asarray(inputs["dist_emb"], f32) @ np.asarray(inputs["dist_W"], f32).T
         + np.asarray(inputs["dist_b"], f32))
    offs = np.arange(-511, 512)
    bi = np.searchsorted(BINS, np.abs(offs), side="right") - 1
    Dfull = D[np.where(offs > 0, bi, bi + NB)]          # [1023, H]
    DfullT = Dfull.T                                     # [H, 1023]

    iotar = np.tile(np.arange(128, dtype=f32), (128, 1))
    mask = np.zeros((128, 2), f32)
    for mi, s in enumerate((K_WARM - 9, K_WARM - 1)):
        c = np.arange(64)
        v = ((8 * c + s) > (K_WARM - 1)).astype(f32)
        mask[0:64, mi] = v
        mask[64:128, mi] = v

    base = {
        "embT_f": embT_f, "embT_b": embT_b,
        "wih0T": wih0T, "whh0T": whh0T, "wih1T": wih1T, "whh1T": whh1T,
        "projT": projT, "wrep_in": wrep, "wrepT_in": wrepT,
        "iotar_in": iotar, "mask_in": mask,
    }

    ah = np.asarray(inputs["arc_head"]).astype(np.int64)
    am = np.asarray(inputs["arc_mod"]).astype(np.int64)
    sh_i = np.asarray(inputs["sib_head"]).astype(np.int64)
    sm_i = np.asarray(inputs["sib_mod"]).astype(np.int64)
    ss_i = np.asarray(inputs["sib_sib"]).astype(np.int64)

    # ---- global tile layouts (uniform across cores; program depends on them)
    # sibs are sharded BY COMBO: combo c's parts split evenly over cores, each
    # core gets ceil(cnt_g[c]/(128*NC)) tiles for combo c.
    sh_p, sm_p, ss_p = PERM[sh_i], PERM[sm_i], PERM[ss_i]
    combo_g = (sh_p // 128) * 16 + (sm_p // 128) * 4 + (ss_p // 128)
    cnt_g = np.bincount(combo_g, minlength=64)
    sib_tpc = -(-cnt_g // (128 * NC))              # tiles per combo per core
    sib_combos = [c for c in range(64) for _ in range(sib_tpc[c])]
    if len(sib_combos) % 2:
        sib_combos.append(int(np.argmax(sib_tpc == 0)) if (sib_tpc == 0).any()
                          else 0)
        sib_pad = 1
    else:
        sib_pad = 0
    n_sib_tile = len(sib_combos)
    sib_tile_off = np.zeros(65, np.int64)          # first tile of each combo
    np.cumsum(sib_tpc, out=sib_tile_off[1:65])
    sib_ids_by_combo = [np.nonzero(combo_g == c)[0] for c in range(64)]

    # arcs stay h-sharded (core owns a 64-row table slice); buckets are
    # (table-half of local h) x (m 64-chunk); bucket tile counts take the
    # max over cores so the layout is core-uniform.  The half-0 prefix is
    # kept even so arc PAIRS never straddle the half boundary.
    core_of = ah // 64
    arc_ids_core = [np.nonzero(core_of == core)[0] for core in range(NC)]
    cnt_ab = np.zeros((NC, 16), np.int64)
    for core in range(NC):
        ids = arc_ids_core[core]
        ab = ((ah[ids] - 64 * core) // 32) * 8 + am[ids] // 64
        cnt_ab[core] = np.bincount(ab, minlength=16)
    arc_tpb = (-(-cnt_ab.max(axis=0) // 128)).astype(np.int64)
    if arc_tpb[:8].sum() % 2:
        arc_tpb[7] += 1
    if arc_tpb.sum() % 2:
        arc_tpb[15] += 1
    arc_buckets = [b for b in range(16) for _ in range(arc_tpb[b])]
    n_arc_tile = len(arc_buckets)
    arc_tile_off = np.zeros(17, np.int64)
    np.cumsum(arc_tpb, out=arc_tile_off[1:17])

    in_maps = []
    meta = {"arc_slots": [], "sib_slots": [],
            "sib_combos": sib_combos, "arc_buckets": arc_buckets}
    for core in range(NC):
        m = dict(base)
        # per-core D window (transposed): cols [448-64c, 1023-64c), zero-pad to 576
        win = np.zeros((512, 576), f32)
        win[:, :575] = DfullT[:, 448 - 64 * core:1023 - 64 * core]
        m["dwin_in"] = bf(win.reshape(4, 128, 576).transpose(1, 0, 2))
        hsel = np.zeros((512, 64), f32)
        hsel[PERM[64 * core + np.arange(64)], np.arange(64)] = 1.0
        m["hsel_in"] = bf(hsel.reshape(4, 128, 64).transpose(1, 0, 2))

        # arcs owned by this core (h in [64c, 64c+64))
        ids = arc_ids_core[core]
        ab = ((ah[ids] - 64 * core) // 32) * 8 + am[ids] // 64
        arc_slot = np.full(n_arc_tile * 128, -1, np.int64)
        order_a = np.argsort(ab, kind="stable")
        pos = 0
        for b in range(16):
            n = cnt_ab[core][b]
            s0 = arc_tile_off[b] * 128
            arc_slot[s0:s0 + n] = ids[order_a[pos:pos + n]]
            pos += n
        arc_rows = np.zeros((n_arc_tile, 128), np.int64)
        arc_mcol = np.zeros((128, n_arc_tile), f32)
        for t in range(n_arc_tile):
            sel = arc_slot[t * 128:(t + 1) * 128]
            valid = sel >= 0
            b = arc_buckets[t]
            arc_rows[t, valid] = ah[sel[valid]] - 64 * core
            arc_mcol[valid, t] = am[sel[valid]] - 64 * (b % 8)
            # invalid slots must still one-hot a row inside this bucket's half
            arc_rows[t, ~valid] = 32 * (b // 8)
        assert arc_rows.min() >= 0 and arc_rows.max() < 64
        aoh = np.zeros((n_arc_tile // 2, 64, 256), f32)
        avals = arc_rows.reshape(n_arc_tile // 2, 2, 128)
        acols = (np.arange(2)[:, None] * 128 + np.arange(128)[None, :])
        aoh[np.arange(n_arc_tile // 2)[:, None, None], avals, acols[None]] = 1
        m["arc_oh_in"] = aoh
        m["arcm_in"] = arc_mcol
        meta["arc_slots"].append(arc_slot)

        # sibs: this core's share of each combo, packed into the combo's tiles
        sib_slot = np.full(n_sib_tile * 128, -1, np.int64)
        for c in range(64):
            gids = sib_ids_by_combo[c]
            n = len(gids)
            base_n, rem = divmod(n, NC)
            lo = core * base_n + min(core, rem)
            hi = lo + base_n + (1 if core < rem else 0)
            part = gids[lo:hi]
            s0 = sib_tile_off[c] * 128
            assert len(part) <= sib_tpc[c] * 128
            sib_slot[s0:s0 + len(part)] = part
        idx_rows = np.zeros((n_sib_tile, 3, 128), np.int64)
        for t in range(n_sib_tile):
            c = sib_combos[t]
            hc, mc_, sc_ = c // 16, (c // 4) % 4, c % 4
            sel = sib_slot[t * 128:(t + 1) * 128]
            valid = sel >= 0
            sv = np.where(valid, sel, 0)
            idx_rows[t, 0] = np.where(valid, sh_p[sv] - 128 * hc, 0)
            idx_rows[t, 1] = np.where(valid, sm_p[sv] - 128 * mc_, 0)
            idx_rows[t, 2] = np.where(valid, ss_p[sv] - 128 * sc_, 0)
        assert idx_rows.max() < 128 and idx_rows.min() >= 0
        soh = np.zeros((n_sib_tile // 2, 128, 768), _BF)
        svals = idx_rows.reshape(n_sib_tile // 2, 2, 3, 128)
        scols = (np.arange(2)[:, None, None] * 384
                 + np.arange(3)[None, :, None] * 128
                 + np.arange(128)[None, None, :])
        soh[np.arange(n_sib_tile // 2)[:, None, None, None], svals,
            scols[None]] = 1
        # padding tile shares a real combo's one-hot slot: zero it out
        if sib_pad:
            soh[-1, :, 384:768] = 0
        m["sib_oh_in"] = soh
        meta["sib_slots"].append(sib_slot)
        in_maps.append(m)
    return in_maps, meta


LAST_EXEC_NS = None


def kernel(**inputs):
    global LAST_EXEC_NS
    _install_ntff_hook()
    from concourse.bass_utils import run_bass_kernel_spmd

    in_maps, meta = _host_prepare(inputs)
    nc = _get_program(meta["sib_combos"], meta["arc_buckets"])
    import os

    trace = os.environ.get("KERNEL_TRACE", "0") == "1"
    res = run_bass_kernel_spmd(nc, in_maps, list(range(NC)), trace=trace)
    LAST_EXEC_NS = res.exec_time_ns
    _CACHE["res"] = res
    n_sib_tile = len(meta["sib_combos"])
    arc_scores = np.zeros(A, np.float32)
    sib_scores = np.zeros(ASIB, np.float32)
    for core in range(NC):
        sc = np.asarray(res.results[core]["scores_out"])  # [128, n_tile]
        sib_flat = sc[:, :n_sib_tile].T.reshape(-1)
        sib_slot = meta["sib_slots"][core]                # global sib ids
        valid = sib_slot >= 0
        sib_scores[sib_slot[valid]] = sib_flat[valid]

        arc_flat = sc[:, n_sib_tile:].T.reshape(-1)
        arc_slot = meta["arc_slots"][core]                # global arc ids
        valid = arc_slot >= 0
        arc_scores[arc_slot[valid]] = arc_flat[valid]
    return np.concatenate([arc_scores, sib_scores])



# revision 90
# speedup vs baseline: 1.1834x; 1.0911x over previous
"""Trainium2 Bass kernel for nn_DependencyNeuralModel (dependency parser scorer).

v2 design (8 NeuronCores, SPMD):
  Encoder: 2-layer BiLSTM over S=512, replicated on every core, chunk-parallel
    (64 chunks x 2 dirs advance lock-step as 128 rows through the PE).
    K_WARM=16 warmup steps; gate order repacked to [i,f,o,g] so the i/f
    half of the recurrent GEMM can overlap the o/g half's activations.
  Arc scores: score(h,m) depends only on the (h,m) pair (dist is a function
    of m-h), so each core builds the 64-row slice of the full SxS score
    table it owns (h sharded), in a transposed layout where the dist term
    is a contiguous slice of a host-built [H, 1023] offset table and the
    head term is a per-partition activation-fused bias.  The per-arc gather
    is then a single GPSIMD ap_gather of scalar (pair) entries from the
    partition-replicated table; host picks the parity lane and unsorts.
  Sib scores: part-sharded; host sorts each core's 16384 parts by the
    (head,mod,sib) 128-chunk combo (64 combos x 3 static tiles), so each
    128-part tile needs only 3 one-hot gather matmuls instead of 12.
Host does only index/layout preparation and final unshard.
"""
import sys
import types

import numpy as np

sys.path.insert(0, "/opt/trn_rl_repo")

import concourse.bass as bass
import concourse.mybir as mybir
from concourse.tile import TileContext
from concourse.masks import make_identity

S = 512
H = 512
A = 262144
ASIB = 131072
NB = 17
L = 8
K_WARM = 14
NSTEP = K_WARM + L  # 24
NC = 8
F32 = mybir.dt.float32
BF16 = mybir.dt.bfloat16
U16 = mybir.dt.uint16
BINS = np.array(list(range(10)) + list(range(10, 40, 5)) + [40], dtype=np.int64)

GPERM = np.r_[0:1024, 1536:2048, 1024:1536]  # gate reorder i,f,g,o -> i,f,o,g


def _install_ntff_hook():
    if "antenv.axon_hooks" in sys.modules:
        return
    mod = types.ModuleType("antenv.axon_hooks")
    state = {"hook": None, "tried": False}

    def set_axon_ntff_profile_hook(hook):
        state["hook"] = hook

    def get_axon_ntff_profile_hook():
        if state["hook"] is None and not state["tried"]:
            state["tried"] = True
            try:
                from trn_agent_boot.trn_boot import _ntff_profile_via_ctypes

                state["hook"] = _ntff_profile_via_ctypes("/opt/axon/libaxon_pjrt.so")
            except Exception:
                state["hook"] = None
        return state["hook"]

    mod.set_axon_ntff_profile_hook = set_axon_ntff_profile_hook
    mod.get_axon_ntff_profile_hook = get_axon_ntff_profile_hook
    import antenv

    antenv.axon_hooks = mod
    sys.modules["antenv.axon_hooks"] = mod


def _legalize_waits(nc):
    """This walrus accepts at most one semaphore wait per instruction;
    split extra waits onto same-engine NOPs placed just before."""
    ctr = [0]
    for f in nc.m.functions:
        for blk in f.blocks:
            out = []
            dirty = False
            for ins in blk.instructions:
                si = ins.sync_info
                if si is not None and si.on_wait and len(si.on_wait) > 1:
                    waits = list(si.on_wait)
                    for w in waits[:-1]:
                        ctr[0] += 1
                        nop = mybir.InstNoOp(name=f"waitfix-{ctr[0]}")
                        nop.engine = ins.engine
                        nop.sync_info = mybir.SyncInfo(on_wait=[w], on_update=[])
                        out.append(nop)
                    ins.sync_info = mybir.SyncInfo(
                        on_wait=[waits[-1]],
                        on_update=list(si.on_update) if si.on_update else [],
                    )
                    dirty = True
                out.append(ins)
            if dirty:
                blk.instructions = out
    return nc


def _lstm_layer(nc, tc, ident, mask_sb, whhT_dram, wx_dram, dstT, dstTrev,
                whh_pre=None):
    """One BiLSTM layer, chunk-parallel.  B=128 rows: partitions 0:64 are
    dir0 chunks, 64:128 dir1 chunks.  Gate columns are [i,f,o,g]; the
    i/f half of the recurrent GEMM is emitted first so its activations
    overlap the o/g half's matmuls.  Output states are written directly
    into the feature-major [128, 9, 512] tiles dstT (and optionally the
    position-reversed dstTrev) from the per-step transposed state."""
    import contextlib

    with contextlib.ExitStack() as ctx:
        sb = ctx.enter_context(tc.tile_pool(name="lstm_sb", bufs=4))
        cold = ctx.enter_context(tc.tile_pool(name="lstm_cold", bufs=1))
        st = ctx.enter_context(tc.tile_pool(name="lstm_state", bufs=1))
        ps1 = ctx.enter_context(tc.tile_pool(name="lstm_ps1", bufs=1, space="PSUM"))
        ps2 = ctx.enter_context(tc.tile_pool(name="lstm_ps2", bufs=1, space="PSUM"))
        pst = ctx.enter_context(tc.tile_pool(name="lstm_pst", bufs=1, space="PSUM"))

        if whh_pre is not None:
            whh_sb = whh_pre
        else:
            whh_sb = st.tile([128, 4, 2, 2048], BF16)
            nc.sync.dma_start(
                whh_sb.rearrange("p a b c -> p (a b c)"),
                whhT_dram.rearrange("p a b c -> p (a b c)"),
            )
        h_t = st.tile([128, 4, 128], BF16)  # h transposed: [k-part, kc, b]
        c_st = st.tile([128, 512], BF16)    # [b, k]
        nc.vector.memset(h_t.rearrange("p a b -> p (a b)"), 0.0)
        nc.vector.memset(c_st[:], 0.0)

        def fetch_wx(s):
            wx = sb.tile([128, 2048], BF16, tag="wx")
            for d in range(2):
                nc.sync.dma_start(
                    wx[d * 64:(d + 1) * 64, :], wx_dram[d, s:s + 505:8, :]
                )
            return wx

        def emit_ident(wx):
            """identity-injection matmuls; emitted during the previous
            step's tail so the PE fills otherwise-idle cycles."""
            g01 = ps1.tile([128, 1024], F32, tag="g01")
            g23 = ps2.tile([128, 1024], F32, tag="g23")
            for half, gps in ((0, g01), (1, g23)):
                for ng in range(2):
                    col = (half * 2 + ng) * 512
                    for d in range(2):
                        bs = slice(d * 64, (d + 1) * 64)
                        nc.tensor.matmul(
                            gps[bs, ng * 512:(ng + 1) * 512],
                            lhsT=ident[:, bs],
                            rhs=wx[:, col:col + 512],
                            start=True, stop=False,
                        )
            return g01, g23

        wxq = {0: fetch_wx(0), 1: fetch_wx(1), 2: fetch_wx(2)}
        nxt = emit_ident(wxq.pop(0))
        for s in range(NSTEP):
            # recurrent half of the gates GEMM.  d0 writes psum rows 0:64
            # (array cols 0-63), d1 rows 64:128 (cols 64-127); adjacent
            # d0/d1 matmuls run concurrently as (128,64) column tiles.
            g01, g23 = nxt
            for half, gps in ((0, g01), (1, g23)):
                for ng in range(2):
                    col = (half * 2 + ng) * 512
                    for kc in range(4):
                        for d in range(2):
                            bs = slice(d * 64, (d + 1) * 64)
                            nc.tensor.matmul(
                                gps[bs, ng * 512:(ng + 1) * 512],
                                lhsT=h_t[:, kc, bs],
                                rhs=whh_sb[:, kc, d, col:col + 512],
                                start=False,
                                stop=(kc == 3),
                            )
            sig_if = cold.tile([128, 1024], BF16, tag="sif")
            nc.scalar.activation(sig_if[:], g01[:],
                                 mybir.ActivationFunctionType.Sigmoid)
            tanh_g = cold.tile([128, 512], BF16, tag="tg")
            nc.scalar.activation(tanh_g[:], g23[:, 512:1024],
                                 mybir.ActivationFunctionType.Tanh)
            sig_o = cold.tile([128, 512], BF16, tag="so")
            nc.scalar.activation(sig_o[:], g23[:, 0:512],
                                 mybir.ActivationFunctionType.Sigmoid)
            t1 = cold.tile([128, 512], BF16, tag="t1")
            nc.vector.tensor_mul(t1[:], sig_if[:, 512:1024], c_st[:])
            t2 = cold.tile([128, 512], BF16, tag="t2")
            nc.vector.tensor_mul(t2[:], sig_if[:, 0:512], tanh_g[:])
            nc.vector.tensor_add(c_st[:], t1[:], t2[:])
            tch = cold.tile([128, 512], BF16, tag="tch")
            nc.scalar.activation(tch[:], c_st[:], mybir.ActivationFunctionType.Tanh)
            h_new = cold.tile([128, 512], BF16, tag="h")
            nc.vector.tensor_mul(h_new[:], sig_o[:], tch[:])
            if s in (K_WARM - 9, K_WARM - 1):
                mi = {K_WARM - 9: 0, K_WARM - 1: 1}[s]
                nc.vector.tensor_scalar_mul(h_new[:], h_new[:], mask_sb[:, mi:mi + 1])
                nc.vector.tensor_scalar_mul(c_st[:], c_st[:], mask_sb[:, mi:mi + 1])
            if s + 3 < NSTEP:
                wxq[s + 3] = fetch_wx(s + 3)
            if s + 1 < NSTEP:
                # next step's identity matmuls go into the PE queue BEFORE
                # this step's transposes: they have no h dependency and run
                # while the tail above executes.
                nxt = emit_ident(wxq.pop(s + 1))
            tp = pst.tile([128, 4, 128], BF16, tag="tr_ps")
            for kc in range(4):
                nc.tensor.transpose(tp[:, kc, :],
                                    h_new[:, kc * 128:(kc + 1) * 128], ident[:])
            nc.vector.tensor_copy(h_t.rearrange("p a b -> p (a b)"),
                                  tp.rearrange("p a b -> p (a b)"))
            if s >= K_WARM:
                o = s - K_WARM
                # scatter this step's transposed states into the
                # feature-major output tiles, which use the PERMUTED
                # position order p' = (pos%8)*64 + pos//8 so every write
                # is a contiguous 64-column block.  dir0 chunk c is
                # position 8c+o -> block o; dir1 (backward) chunk c is
                # position 511-(8c+o) -> block 7-o, chunk axis reversed.
                nc.vector.tensor_copy(dstT[:, 0:4, o * 64:(o + 1) * 64],
                                      tp[:, :, 0:64])
                nc.vector.tensor_copy(
                    dstT[:, 4:8, (7 - o) * 64:(8 - o) * 64],
                    tp[:, :, 127:63:-1])
                if dstTrev is not None:
                    nc.vector.tensor_copy(
                        dstTrev[:, 0:4, (7 - o) * 64:(8 - o) * 64],
                        tp[:, :, 63::-1])
                    nc.vector.tensor_copy(
                        dstTrev[:, 4:8, o * 64:(o + 1) * 64],
                        tp[:, :, 64:128])


def _input_gemm(nc, tc, lhsT_tiles, wihT_dram, wx_dram, nk, klast, pre=None):
    """WX[d] = lhsT_d.T @ wihT[d] -> wx_dram[d, K_WARM:K_WARM+512, :].
    lhsT_tiles: per-dir tile [128, nk, 512] in SBUF ([feat-part, chunk, pos]).
    nk chunks; last chunk has klast valid rows.  If pre is given it is an
    SBUF-resident [128, nk, 2, 2048] copy of the weights (prefetched long
    before, so this GEMM issues no DMA reads at all)."""
    import contextlib

    with contextlib.ExitStack() as ctx:
        sb = ctx.enter_context(tc.tile_pool(name="ig_sb", bufs=6))
        ps = ctx.enter_context(tc.tile_pool(name="ig_ps", bufs=2, space="PSUM"))
        for d in range(2):
            lhsT = lhsT_tiles[d]
            for ngc in range(4):
                acc4 = ps.tile([128, 4, 512], F32, tag="acc4")
                for kc in range(nk):
                    kk = 128 if kc < nk - 1 else klast
                    if pre is not None:
                        rhs_ap = pre[:kk, kc, d, ngc * 512:(ngc + 1) * 512]
                    else:
                        rhs = sb.tile([128, 512], wihT_dram.dtype, tag="rhs")
                        nc.sync.dma_start(
                            rhs[:kk, :],
                            wihT_dram[kc * 128:kc * 128 + kk, d,
                                      ngc * 512:(ngc + 1) * 512],
                        )
                        rhs_ap = rhs[:kk, :]
                    for mc in range(4):
                        nc.tensor.matmul(
                            acc4[:, mc, :],
                            lhsT=lhsT[:kk, kc, mc * 128:(mc + 1) * 128],
                            rhs=rhs_ap,
                            start=(kc == 0),
                            stop=(kc == nk - 1),
                        )
                osb = sb.tile([128, 4, 512], BF16, tag="osb")
                nc.scalar.activation(
                    osb.rearrange("p a b -> p (a b)"),
                    acc4.rearrange("p a b -> p (a b)"),
                    mybir.ActivationFunctionType.Copy)
                # lhsT columns are in permuted position order
                # p' = o*64 + c (o = 2*mc + a); scatter rows back to the
                # natural sliding-window rows 8c + o of wx_dram.
                for mc in range(4):
                    for a in range(2):
                        r0 = K_WARM + 2 * mc + a
                        nc.sync.dma_start(
                            wx_dram[d, r0:r0 + 505:8,
                                    ngc * 512:(ngc + 1) * 512],
                            osb[a * 64:(a + 1) * 64, mc, :],
                        )


def _build(nc, sib_combos, arc_buckets):
    dt = F32
    n_sib_tile = len(sib_combos)       # even
    n_arc_tile = len(arc_buckets)      # even
    n_tile = n_sib_tile + n_arc_tile
    embT_f = nc.dram_tensor("embT_f", [128, 3, 512], BF16, kind="ExternalInput")
    embT_b = nc.dram_tensor("embT_b", [128, 3, 512], BF16, kind="ExternalInput")
    wih0T = nc.dram_tensor("wih0T", [384, 2, 2048], BF16, kind="ExternalInput")
    whh0T = nc.dram_tensor("whh0T", [128, 4, 2, 2048], BF16, kind="ExternalInput")
    wih1T = nc.dram_tensor("wih1T", [1152, 2, 2048], BF16, kind="ExternalInput")
    whh1T = nc.dram_tensor("whh1T", [128, 4, 2, 2048], BF16, kind="ExternalInput")
    projT = nc.dram_tensor("projT", [1152, 2560], BF16, kind="ExternalInput")
    dwin_in = nc.dram_tensor("dwin_in", [128, 4, 576], BF16, kind="ExternalInput")
    hsel_in = nc.dram_tensor("hsel_in", [128, 4, 64], BF16, kind="ExternalInput")
    wrep_in = nc.dram_tensor("wrep_in", [128, 512], BF16, kind="ExternalInput")
    wrepT_in = nc.dram_tensor("wrepT_in", [128, 4, 128], BF16, kind="ExternalInput")
    sib_oh_in = nc.dram_tensor("sib_oh_in", [n_sib_tile // 2, 128, 768], BF16,
                               kind="ExternalInput")
    arc_oh_in = nc.dram_tensor("arc_oh_in", [n_arc_tile // 2, 64, 256], dt,
                               kind="ExternalInput")
    arcm_in = nc.dram_tensor("arcm_in", [128, n_arc_tile], dt,
                             kind="ExternalInput")
    iotar_in = nc.dram_tensor("iotar_in", [128, 128], dt, kind="ExternalInput")
    mask_in = nc.dram_tensor("mask_in", [128, 2], dt, kind="ExternalInput")
    scores_out = nc.dram_tensor("scores_out", [128, n_tile], dt,
                                kind="ExternalOutput")

    wx0 = nc.dram_tensor("wx0", [2, 544, 2048], BF16)
    tdram = nc.dram_tensor("tdram", [64, 512], F32)
    wx1 = nc.dram_tensor("wx1", [2, 544, 2048], BF16)
    tdram = nc.dram_tensor("tdram", [64, 512], F32)

    import contextlib

    with TileContext(nc) as tc:
        with contextlib.ExitStack() as ctx:
            const = ctx.enter_context(tc.tile_pool(name="const", bufs=1))
            big = ctx.enter_context(tc.tile_pool(name="big", bufs=1))

            ident = const.tile([128, 128], BF16)
            make_identity(nc, ident[:])
            mask_sb = const.tile([128, 2], dt)
            nc.sync.dma_start(mask_sb[:], mask_in[:])
            one_row = const.tile([1, 512], BF16)
            nc.vector.memset(one_row[:], 1.0)
            wrep_sb = const.tile([128, 512], BF16)
            wrepT_sb = const.tile([128, 4, 128], BF16)
            dwin_sb = const.tile([128, 4, 576], BF16)
            hsel_sb = const.tile([128, 4, 64], BF16)
            iota_row = const.tile([128, 128], dt)
            arcm_sb = const.tile([128, n_arc_tile], dt)

            # zero-pad warmup rows of WX buffers
            with tc.tile_pool(name="zp", bufs=1) as zp:
                zrow = zp.tile([64, 2048], BF16)
                nc.vector.memset(zrow[:], 0.0)
                for wxd in (wx0, wx1):
                    for d in range(2):
                        nc.sync.dma_start(wxd[d, 0:K_WARM, :], zrow[0:K_WARM, :])
                        nc.sync.dma_start(wxd[d, K_WARM + 512:544, :],
                                          zrow[0:32 - K_WARM, :])

            # ---- layer 0 (streams states into x1T / x1Trev) ----
            x1T = big.tile([128, 9, 512], BF16, tag="x1T")
            x1Trev = big.tile([128, 9, 512], BF16, tag="x1Trev")
            for dst in (x1T, x1Trev):
                nc.vector.memset(dst[:, 8, :], 0.0)
                nc.vector.tensor_copy(dst[0:1, 8, :], one_row[:])

            with tc.tile_pool(name="w1pre", bufs=1) as w1p:
                # ---- WX0 + weight prefetch ----
                # the sync queue stalls at WX0's first sem-gated output
                # write, so any DMA issued after WX0 starts ~30us late.
                # Issue embeddings first, then layer-0 recurrent weights,
                # then the wih1T prefetch -- all BEFORE the WX0 GEMM body.
                with tc.tile_pool(name="emb_sb", bufs=1) as emb_pool:
                    ef = emb_pool.tile([128, 3, 512], BF16)
                    nc.sync.dma_start(ef.rearrange("p a b -> p (a b)"),
                                      embT_f.rearrange("p a b -> p (a b)"))
                    eb = emb_pool.tile([128, 3, 512], BF16)
                    nc.sync.dma_start(eb.rearrange("p a b -> p (a b)"),
                                      embT_b.rearrange("p a b -> p (a b)"))
                    whh_sb = big.tile([128, 4, 2, 2048], BF16, tag="whh")
                    nc.sync.dma_start(
                        whh_sb.rearrange("p a b c -> p (a b c)"),
                        whh0T.rearrange("p a b c -> p (a b c)"))
                    _input_gemm(nc, tc, [ef, eb], wih0T, wx0, 3, 128)
                # wih1T prefetch: queued behind WX0's sync-stalls is fine --
                # it only has to complete before WX1, ~280us later.
                w1sb = w1p.tile([128, 9, 2, 2048], BF16, tag="w1")
                for kc in range(9):
                    for d in range(2):
                        nc.sync.dma_start(
                            w1sb[:, kc, d, :],
                            wih1T[kc * 128:(kc + 1) * 128, d, :])

                _lstm_layer(nc, tc, ident, mask_sb, whh0T, wx0, x1T, x1Trev,
                            whh_pre=whh_sb)

                # reload the shared recurrent-weight tile with layer 1's
                # weights; transfers during WX1 so layer 1 starts instantly
                nc.sync.dma_start(
                    whh_sb.rearrange("p a b c -> p (a b c)"),
                    whh1T.rearrange("p a b c -> p (a b c)"))

                # scoring-only constants load behind the startup-critical DMAs
                nc.sync.dma_start(wrep_sb[:], wrep_in[:])
                nc.sync.dma_start(wrepT_sb.rearrange("p a b -> p (a b)"),
                                  wrepT_in.rearrange("p a b -> p (a b)"))
                nc.sync.dma_start(dwin_sb.rearrange("p a b -> p (a b)"),
                                  dwin_in.rearrange("p a b -> p (a b)"))
                nc.sync.dma_start(hsel_sb.rearrange("p a b -> p (a b)"),
                                  hsel_in.rearrange("p a b -> p (a b)"))
                nc.sync.dma_start(iota_row[:], iotar_in[:])
                nc.sync.dma_start(arcm_sb[:], arcm_in[:])

                # ---- WX1 ----
                _input_gemm(nc, tc, [x1T, x1Trev], wih1T, wx1, 9, 1, pre=w1sb)

            # ---- layer 1 (streams states into stT) ----
            stT = big.tile([128, 9, 512], BF16, tag="x1Trev")  # reuse slot
            nc.vector.memset(stT[:, 8, :], 0.0)
            nc.vector.tensor_copy(stT[0:1, 8, :], one_row[:])
            ppre_pool = ctx.enter_context(tc.tile_pool(name="ppre", bufs=1))
            ppre = ppre_pool.tile([128, 9, 2560], BF16)
            for kc in range(9):
                nc.sync.dma_start(ppre[:, kc, :],
                                  projT[kc * 128:(kc + 1) * 128, :])
            _lstm_layer(nc, tc, ident, mask_sb, whh1T, wx1, stT, None,
                        whh_pre=whh_sb)

            # ---- pos-major projection tables (head + 3 sib; skip mod) ----
            tables_sb = big.tile([128, 4, 2560], BF16, tag="tables")
            with contextlib.ExitStack() as c2:
                sb2 = c2.enter_context(tc.tile_pool(name="tb_sb", bufs=6))
                with tc.tile_pool(name="tb_ps4", bufs=2, space="PSUM") as ps4:
                    for ngc in (0,):
                        acc4 = ps4.tile([128, 4, 512], dt, tag="acc4")
                        for kc in range(9):
                            kk = 128 if kc < 8 else 1
                            for mc in range(4):
                                nc.tensor.matmul(
                                    acc4[:, mc, :],
                                    lhsT=stT[:kk, kc, mc * 128:(mc + 1) * 128],
                                    rhs=ppre[:kk, kc,
                                             ngc * 512:(ngc + 1) * 512],
                                    start=(kc == 0),
                                    stop=(kc == 8),
                                )
                        for mc in range(4):
                            nc.scalar.activation(
                                tables_sb[:, mc, ngc * 512:(ngc + 1) * 512],
                                acc4[:, mc, :],
                                mybir.ActivationFunctionType.Copy)
                ps2 = c2.enter_context(tc.tile_pool(name="tb_ps", bufs=2,
                                                    space="PSUM"))

                # ---- transposed mod table M_T[j, m] ----
                mTp = big.tile([128, 4, 512], BF16, tag="mTp")
                for jc in range(4):
                    acc = ps2.tile([128, 512], dt, tag="acc")
                    for kc in range(8):
                        nc.tensor.matmul(
                            acc[:],
                            lhsT=ppre[:, kc,
                                      512 + jc * 128:512 + (jc + 1) * 128],
                            rhs=stT[:, kc, :],
                            start=(kc == 0), stop=(kc == 7),
                        )
                    nc.scalar.activation(mTp[:, jc, :], acc[:],
                                         mybir.ActivationFunctionType.Copy)
                # un-permute m columns to natural position order (one-time)
                mT = big.tile([128, 4, 512], BF16, tag="mT")
                for o in range(8):
                    nc.vector.tensor_copy(mT[:, :, o::8],
                                          mTp[:, :, o * 64:(o + 1) * 64])

                # ---- H window: hwin[j, hl] = heads[64c+hl, j] ----
                hwin = big.tile([128, 4, 64], dt, tag="hwin")
                for jc in range(4):
                    acc = ps2.tile([128, 64], dt, tag="acch")
                    for kc in range(4):
                        nc.tensor.matmul(
                            acc[:],
                            lhsT=tables_sb[:, kc, jc * 128:(jc + 1) * 128],
                            rhs=hsel_sb[:, kc, :],
                            start=(kc == 0), stop=(kc == 3),
                        )
                    nc.vector.tensor_copy(hwin[:, jc, :], acc[:])

                # sib projection tables AFTER mT/hwin: the scoring rows only
                # need mT+hwin, so their ACT/DVE pipeline starts while these
                # PE-only GEMMs still stream.
                with tc.tile_pool(name="tb_ps5", bufs=1, space="PSUM") as ps5:
                    for ngc in (2, 3, 4):
                        acc4 = ps5.tile([128, 4, 512], dt, tag="acc4b")
                        for kc in range(9):
                            kk = 128 if kc < 8 else 1
                            for mc in range(4):
                                nc.tensor.matmul(
                                    acc4[:, mc, :],
                                    lhsT=stT[:kk, kc, mc * 128:(mc + 1) * 128],
                                    rhs=ppre[:kk, kc,
                                             ngc * 512:(ngc + 1) * 512],
                                    start=(kc == 0),
                                    stop=(kc == 8),
                                )
                        for mc in range(4):
                            nc.scalar.activation(
                                tables_sb[:, mc, ngc * 512:(ngc + 1) * 512],
                                acc4[:, mc, :],
                                mybir.ActivationFunctionType.Copy)

            # ---- scoring ----
            scores_sb = big.tile([128, n_tile], dt, tag="scores")
            with contextlib.ExitStack() as c3:
                sb3 = c3.enter_context(tc.tile_pool(name="sc_sb", bufs=4))
                ps_sacc = c3.enter_context(tc.tile_pool(name="ps_sacc", bufs=2,
                                                        space="PSUM"))

                def sib_pair(p):
                    # sib tiles 2p, 2p+1 with host-uploaded one-hots
                    oh = sb3.tile([128, 2, 3, 128], BF16, tag="oh")
                    nc.sync.dma_start(
                        oh.rearrange("p a b c -> p (a b c)"),
                        sib_oh_in[p, :, :],
                    )
                    sacc2 = ps_sacc.tile([128, 2, 512], dt, tag="sacc2")
                    for h2 in range(2):
                        combo = sib_combos[2 * p + h2]
                        chunks = (combo // 16, (combo // 4) % 4, combo % 4)
                        for g in range(3):
                            nc.tensor.matmul(
                                sacc2[:, h2, :], lhsT=oh[:, h2, g, :],
                                rhs=tables_sb[:, chunks[g],
                                              1024 + g * 512:
                                              1024 + (g + 1) * 512],
                                start=(g == 0), stop=(g == 2),
                            )
                    th2 = sb3.tile([128, 2, 512], BF16, tag="th2")
                    nc.scalar.activation(
                        th2.rearrange("p a b -> p (a b)"),
                        sacc2.rearrange("p a b -> p (a b)"),
                        mybir.ActivationFunctionType.Tanh)
                    for h2 in range(2):
                        junk = sb3.tile([128, 512], BF16, tag="junk")
                        nc.vector.scalar_tensor_tensor(
                            junk[:], th2[:, h2, :], 1.0, wrep_sb[:],
                            op0=mybir.AluOpType.mult,
                            op1=mybir.AluOpType.mult,
                            accum_out=scores_sb[:, 2 * p + h2:2 * p + h2 + 1],
                        )

                table_hm = big.tile([64, 512], dt, tag="table_hm")
                n_sib_pair = n_sib_tile // 2
                n_arc_pair = n_arc_tile // 2
                # arc tiles are bucketed by (table half, m 64-chunk); half-0
                # tiles form an even-length prefix and only need table rows
                # 0:32, so they can interleave while rows 32:63 compute.
                n_h0_pair = sum(1 for b in arc_buckets if b < 8) // 2

                with tc.tile_pool(name="ps_tblw", bufs=2, space="PSUM") as ps_w, \
                     tc.tile_pool(name="ps_arc", bufs=2, space="PSUM") as ps_a:

                    def emit_row(hl):
                        tmp = sb3.tile([128, 4, 512], BF16, tag="tmp")
                        nc.vector.tensor_add(
                            tmp[:], mT[:],
                            dwin_sb[:, :, 63 - hl:63 - hl + 512])
                        tht = sb3.tile([128, 4, 512], BF16, tag="tht")
                        for jc in range(4):
                            nc.scalar.activation(
                                tht[:, jc, :], tmp[:, jc, :],
                                mybir.ActivationFunctionType.Tanh,
                                bias=hwin[:, jc, hl:hl + 1])
                        wps = ps_w.tile([128, 512], dt, tag="wps")
                        for jc in range(4):
                            nc.tensor.matmul(
                                wps[:], lhsT=wrepT_sb[:, jc, :],
                                rhs=tht[:, jc, :],
                                start=(jc == 0), stop=(jc == 3),
                            )
                        trow = sb3.tile([128, 512], dt, tag="trow")
                        nc.vector.tensor_copy(trow[:], wps[:])
                        nc.sync.dma_start(tdram[hl:hl + 1, :], trow[0:1, :])

                    def arc_pair(pa):
                        # arc tiles u=2pa, 2pa+1 with host-uploaded h one-hots
                        u = 2 * pa
                        ohh = sb3.tile([64, 2, 128], dt, tag="ohh")
                        nc.sync.dma_start(
                            ohh.rearrange("p a b -> p (a b)"),
                            arc_oh_in[pa, :, :],
                        )
                        comb = ps_a.tile([128, 2, 64], dt, tag="comb")
                        for h2 in range(2):
                            bucket = arc_buckets[u + h2]
                            mq = bucket % 8
                            nc.tensor.matmul(
                                comb[:, h2, :], lhsT=ohh[:, h2, :],
                                rhs=table_hm[0:64, mq * 64:(mq + 1) * 64],
                                start=True, stop=True,
                            )
                        for h2 in range(2):
                            junk2 = sb3.tile([128, 64], BF16, tag="junk2")
                            nc.vector.scalar_tensor_tensor(
                                junk2[:], iota_row[:, 0:64],
                                arcm_sb[:, u + h2:u + h2 + 1],
                                comb[:, h2, :],
                                op0=mybir.AluOpType.is_equal,
                                op1=mybir.AluOpType.mult,
                                accum_out=scores_sb[:, n_sib_tile + u + h2:
                                                    n_sib_tile + u + h2 + 1],
                            )

                    cur = {"sib": 0, "arc": 0}

                    def sibs(n):
                        k = min(n, n_sib_pair - cur["sib"])
                        for _ in range(k):
                            sib_pair(cur["sib"])
                            cur["sib"] += 1

                    def arcs(n):
                        k = min(n, n_arc_pair - cur["arc"])
                        for _ in range(k):
                            arc_pair(cur["arc"])
                            cur["arc"] += 1

                    # phase A: table rows with a few sib pairs for PE overlap
                    for hl in range(64):
                        emit_row(hl)
                        if hl % 3 == 2:
                            sibs(1)
                    nc.sync.dma_start(table_hm[:], tdram[:])
                    # phase B: remaining sib pairs with arc runs between (runs
                    # amortize the PE mode switch between the 128-row sib
                    # gathers and the 64-row arc gathers)
                    while cur["sib"] < n_sib_pair or cur["arc"] < n_arc_pair:
                        sibs(1)
                        arcs(3)

                nc.sync.dma_start(scores_out[:], scores_sb[:])
    return nc


_CACHE = {}


def _get_program(sib_combos, arc_buckets):
    key = (tuple(sib_combos), tuple(arc_buckets))
    if _CACHE.get("key") != key:
        nc = bass.Bass()
        _build(nc, sib_combos, arc_buckets)
        _legalize_waits(nc)
        _CACHE["nc"] = nc
        _CACHE["key"] = key
    return _CACHE["nc"]


def _host_prepare(inputs):
    import jax.numpy as jnp
    import ml_dtypes
    _BF = ml_dtypes.bfloat16

    def bf(x):
        return np.asarray(jnp.asarray(np.asarray(x, np.float32), jnp.bfloat16))

    f32 = np.float32
    words = np.asarray(inputs["words"]).astype(np.int64)
    tags = np.asarray(inputs["tags"]).astype(np.int64)
    word_emb = np.asarray(inputs["word_emb"], f32)
    tag_emb = np.asarray(inputs["tag_emb"], f32)
    emb = np.concatenate([word_emb[words], tag_emb[tags]], axis=-1)  # [512, 364]
    emb_aug = np.concatenate([emb, np.ones((S, 1), f32)], axis=1)    # [512, 365]

    def packT(x, rows):  # -> [rows(pad), ...] = x.T zero-padded
        out = np.zeros((rows, x.shape[0]), f32)
        out[: x.shape[1]] = x.T
        return out

    # permuted position order used by all feature-major [feat, pos] tiles:
    # column p' holds position IPERM[p'];  PERM[pos] = (pos%8)*64 + pos//8
    PERM = (np.arange(S) % 8) * 64 + np.arange(S) // 8
    IPERM = np.argsort(PERM)

    embT_f = bf(packT(emb_aug, 384)[:, IPERM]
                .reshape(3, 128, 512).transpose(1, 0, 2))
    embT_b = bf(packT(emb_aug[::-1], 384)[:, IPERM]
                .reshape(3, 128, 512).transpose(1, 0, 2))

    def wih_pack(Wih, bih, bhh, kdim, rows):
        out = np.zeros((rows, 2, 4 * H), f32)
        for d in range(2):
            out[:kdim, d] = np.asarray(Wih[d], f32).T[:, GPERM]
            out[kdim, d] = (np.asarray(bih[d], f32) + np.asarray(bhh[d], f32))[GPERM]
        return out

    wih0T = bf(wih_pack(inputs["Wih0"], inputs["bih0"], inputs["bhh0"], 364, 384))
    wih1T = bf(wih_pack(inputs["Wih1"], inputs["bih1"], inputs["bhh1"], 1024, 1152))

    def whh_pack(Whh):
        out = np.zeros((128, 4, 2, 4 * H), f32)
        for d in range(2):
            wt = np.asarray(Whh[d], f32).T[:, GPERM]  # [512 k, 2048 g]
            out[:, :, d, :] = wt.reshape(4, 128, 4 * H).transpose(1, 0, 2)
        return out

    whh0T = bf(whh_pack(inputs["Whh0"]))
    whh1T = bf(whh_pack(inputs["Whh1"]))

    projs = [inputs["head_W"], inputs["mod_W"], inputs["sib_head_W"],
             inputs["sib_mod_W"], inputs["sib_sib_W"]]
    projT = np.zeros((1152, 5 * H), f32)
    for i, W in enumerate(projs):
        projT[:1024, i * H:(i + 1) * H] = np.asarray(W, f32).T
    projT = bf(projT)

    w = np.asarray(inputs["arc_w"], f32).reshape(512)
    wrep = bf(np.broadcast_to(w, (128, 512)))
    wrepT = bf(w.reshape(4, 128).T.reshape(128, 4, 1).repeat(128, axis=2))

    # Dfull[off] = D[distidx(off - 511)], off in [0, 1022]
    D = (np.asarray(inputs["dist_emb"], f32) @ np.asarray(inputs["dist_W"], f32).T
         + np.asarray(inputs["dist_b"], f32))
    offs = np.arange(-511, 512)
    bi = np.searchsorted(BINS, np.abs(offs), side="right") - 1
    Dfull = D[np.where(offs > 0, bi, bi + NB)]          # [1023, H]
    DfullT = Dfull.T                                     # [H, 1023]

    iotar = np.tile(np.arange(128, dtype=f32), (128, 1))
    mask = np.zeros((128, 2), f32)
    for mi, s in enumerate((K_WARM - 9, K_WARM - 1)):
        c = np.arange(64)
        v = ((8 * c + s) > (K_WARM - 1)).astype(f32)
        mask[0:64, mi] = v
        mask[64:128, mi] = v

    base = {
        "embT_f": embT_f, "embT_b": embT_b,
        "wih0T": wih0T, "whh0T": whh0T, "wih1T": wih1T, "whh1T": whh1T,
        "projT": projT, "wrep_in": wrep, "wrepT_in": wrepT,
        "iotar_in": iotar, "mask_in": mask,
    }

    ah = np.asarray(inputs["arc_head"]).astype(np.int64)
    am = np.asarray(inputs["arc_mod"]).astype(np.int64)
    sh_i = np.asarray(inputs["sib_head"]).astype(np.int64)
    sm_i = np.asarray(inputs["sib_mod"]).astype(np.int64)
    ss_i = np.asarray(inputs["sib_sib"]).astype(np.int64)

    # ---- global tile layouts (uniform across cores; program depends on them)
    # sibs are sharded BY COMBO: combo c's parts split evenly over cores, each
    # core gets ceil(cnt_g[c]/(128*NC)) tiles for combo c.
    sh_p, sm_p, ss_p = PERM[sh_i], PERM[sm_i], PERM[ss_i]
    combo_g = (sh_p // 128) * 16 + (sm_p // 128) * 4 + (ss_p // 128)
    cnt_g = np.bincount(combo_g, minlength=64)
    sib_tpc = -(-cnt_g // (128 * NC))              # tiles per combo per core
    sib_combos = [c for c in range(64) for _ in range(sib_tpc[c])]
    if len(sib_combos) % 2:
        sib_combos.append(int(np.argmax(sib_tpc == 0)) if (sib_tpc == 0).any()
                          else 0)
        sib_pad = 1
    else:
        sib_pad = 0
    n_sib_tile = len(sib_combos)
    sib_tile_off = np.zeros(65, np.int64)          # first tile of each combo
    np.cumsum(sib_tpc, out=sib_tile_off[1:65])
    sib_ids_by_combo = [np.nonzero(combo_g == c)[0] for c in range(64)]

    # arcs stay h-sharded (core owns a 64-row table slice); buckets are
    # (table-half of local h) x (m 64-chunk); bucket tile counts take the
    # max over cores so the layout is core-uniform.  The half-0 prefix is
    # kept even so arc PAIRS never straddle the half boundary.
    core_of = ah // 64
    arc_ids_core = [np.nonzero(core_of == core)[0] for core in range(NC)]
    cnt_ab = np.zeros((NC, 16), np.int64)
    for core in range(NC):
        ids = arc_ids_core[core]
        ab = ((ah[ids] - 64 * core) // 32) * 8 + am[ids] // 64
        cnt_ab[core] = np.bincount(ab, minlength=16)
    arc_tpb = (-(-cnt_ab.max(axis=0) // 128)).astype(np.int64)
    if arc_tpb[:8].sum() % 2:
        arc_tpb[7] += 1
    if arc_tpb.sum() % 2:
        arc_tpb[15] += 1
    arc_buckets = [b for b in range(16) for _ in range(arc_tpb[b])]
    n_arc_tile = len(arc_buckets)
    arc_tile_off = np.zeros(17, np.int64)
    np.cumsum(arc_tpb, out=arc_tile_off[1:17])

    in_maps = []
    meta = {"arc_slots": [], "sib_slots": [],
            "sib_combos": sib_combos, "arc_buckets": arc_buckets}
    for core in range(NC):
        m = dict(base)
        # per-core D window (transposed): cols [448-64c, 1023-64c), zero-pad to 576
        win = np.zeros((512, 576), f32)
        win[:, :575] = DfullT[:, 448 - 64 * core:1023 - 64 * core]
        m["dwin_in"] = bf(win.reshape(4, 128, 576).transpose(1, 0, 2))
        hsel = np.zeros((512, 64), f32)
        hsel[PERM[64 * core + np.arange(64)], np.arange(64)] = 1.0
        m["hsel_in"] = bf(hsel.reshape(4, 128, 64).transpose(1, 0, 2))

        # arcs owned by this core (h in [64c, 64c+64))
        ids = arc_ids_core[core]
        ab = ((ah[ids] - 64 * core) // 32) * 8 + am[ids] // 64
        arc_slot = np.full(n_arc_tile * 128, -1, np.int64)
        order_a = np.argsort(ab, kind="stable")
        pos = 0
        for b in range(16):
            n = cnt_ab[core][b]
            s0 = arc_tile_off[b] * 128
            arc_slot[s0:s0 + n] = ids[order_a[pos:pos + n]]
            pos += n
        arc_rows = np.zeros((n_arc_tile, 128), np.int64)
        arc_mcol = np.zeros((128, n_arc_tile), f32)
        for t in range(n_arc_tile):
            sel = arc_slot[t * 128:(t + 1) * 128]
            valid = sel >= 0
            b = arc_buckets[t]
            arc_rows[t, valid] = ah[sel[valid]] - 64 * core
            arc_mcol[valid, t] = am[sel[valid]] - 64 * (b % 8)
            # invalid slots must still one-hot a row inside this bucket's half
            arc_rows[t, ~valid] = 32 * (b // 8)
        assert arc_rows.min() >= 0 and arc_rows.max() < 64
        aoh = np.zeros((n_arc_tile // 2, 64, 256), f32)
        avals = arc_rows.reshape(n_arc_tile // 2, 2, 128)
        acols = (np.arange(2)[:, None] * 128 + np.arange(128)[None, :])
        aoh[np.arange(n_arc_tile // 2)[:, None, None], avals, acols[None]] = 1
        m["arc_oh_in"] = aoh
        m["arcm_in"] = arc_mcol
        meta["arc_slots"].append(arc_slot)

        # sibs: this core's share of each combo, packed into the combo's tiles
        sib_slot = np.full(n_sib_tile * 128, -1, np.int64)
        for c in range(64):
            gids = sib_ids_by_combo[c]
            n = len(gids)
            base_n, rem = divmod(n, NC)
            lo = core * base_n + min(core, rem)
            hi = lo + base_n + (1 if core < rem else 0)
            part = gids[lo:hi]
            s0 = sib_tile_off[c] * 128
            assert len(part) <= sib_tpc[c] * 128
            sib_slot[s0:s0 + len(part)] = part
        idx_rows = np.zeros((n_sib_tile, 3, 128), np.int64)
        for t in range(n_sib_tile):
            c = sib_combos[t]
            hc, mc_, sc_ = c // 16, (c // 4) % 4, c % 4
            sel = sib_slot[t * 128:(t + 1) * 128]
            valid = sel >= 0
            sv = np.where(valid, sel, 0)
            idx_rows[t, 0] = np.where(valid, sh_p[sv] - 128 * hc, 0)
            idx_rows[t, 1] = np.where(valid, sm_p[sv] - 128 * mc_, 0)
            idx_rows[t, 2] = np.where(valid, ss_p[sv] - 128 * sc_, 0)
        assert idx_rows.max() < 128 and idx_rows.min() >= 0
        soh = np.zeros((n_sib_tile // 2, 128, 768), _BF)
        svals = idx_rows.reshape(n_sib_tile // 2, 2, 3, 128)
        scols = (np.arange(2)[:, None, None] * 384
                 + np.arange(3)[None, :, None] * 128
                 + np.arange(128)[None, None, :])
        soh[np.arange(n_sib_tile // 2)[:, None, None, None], svals,
            scols[None]] = 1
        # padding tile shares a real combo's one-hot slot: zero it out
        if sib_pad:
            soh[-1, :, 384:768] = 0
        m["sib_oh_in"] = soh
        meta["sib_slots"].append(sib_slot)
        in_maps.append(m)
    return in_maps, meta


LAST_EXEC_NS = None


def kernel(**inputs):
    global LAST_EXEC_NS
    _install_ntff_hook()
    from concourse.bass_utils import run_bass_kernel_spmd

    in_maps, meta = _host_prepare(inputs)
    nc = _get_program(meta["sib_combos"], meta["arc_buckets"])
    import os

    trace = os.environ.get("KERNEL_TRACE", "0") == "1"
    res = run_bass_kernel_spmd(nc, in_maps, list(range(NC)), trace=trace)
    LAST_EXEC_NS = res.exec_time_ns
    _CACHE["res"] = res
    n_sib_tile = len(meta["sib_combos"])
    arc_scores = np.zeros(A, np.float32)
    sib_scores = np.zeros(ASIB, np.float32)
    for core in range(NC):
        sc = np.asarray(res.results[core]["scores_out"])  # [128, n_tile]
        sib_flat = sc[:, :n_sib_tile].T.reshape(-1)
        sib_slot = meta["sib_slots"][core]                # global sib ids
        valid = sib_slot >= 0
        sib_scores[sib_slot[valid]] = sib_flat[valid]

        arc_flat = sc[:, n_sib_tile:].T.reshape(-1)
        arc_slot = meta["arc_slots"][core]                # global arc ids
        valid = arc_slot >= 0
        arc_scores[arc_slot[valid]] = arc_flat[valid]
    return np.concatenate([arc_scores, sib_scores])

